# revision 1
# baseline (speedup 1.0000x reference)
"""Autoformer forward on 8 Trainium2 NeuronCores, data-parallel over batch."""
import math
import os
import sys

sys.path.insert(0, "/opt/trn_rl_repo")
import numpy as np
from contextlib import ExitStack

import concourse.bass as bass
import concourse.bacc as bacc
import concourse.mybir as mybir
from concourse.tile import TileContext
from concourse.bass_utils import run_bass_kernel_spmd

AL = mybir.AluOpType
AF = mybir.ActivationFunctionType
F32 = mybir.dt.float32
F32R = mybir.dt.float32r
U32 = mybir.dt.uint32
AX = mybir.AxisListType
DVE = mybir.EngineType.DVE

B, SEQ, LAB, PRED = 32, 720, 336, 720
CIN, D, H, DFF, EL, MA = 21, 512, 8, 2048, 2, 25
L1, L2 = SEQ, LAB + PRED            # 720, 1056
LF1, LF2 = L1 // 2 + 1, L2 // 2 + 1  # 361, 529
TOPK = 6
PAD = (MA - 1) // 2                 # 12
NCORES = 8
BC = B // NCORES                    # 4 batches per core
DC = D // 128                       # 4 feature chunks
DFC = DFF // 128                    # 16

TC1 = (L1 + 127) // 128             # 6   seq chunks (enc)
TC2 = (L2 + 127) // 128             # 9   seq chunks (dec)
NFC1 = (LF1 + 127) // 128           # 3   freq chunks (enc)
NFC2 = (LF2 + 127) // 128           # 5   freq chunks (dec)
NT1 = [(0, 360), (360, 360)]
NT2 = [(0, 352), (352, 352), (704, 352)]


def _ft_chunks(lf, nfc):
    return [(i * 128, min(128, lf - i * 128)) for i in range(nfc)]


FT1 = _ft_chunks(LF1, NFC1)
FT2 = _ft_chunks(LF2, NFC2)


# ---------------------------------------------------------------- host consts
def _dft_mats(L, LF):
    t = np.arange(L, dtype=np.float64)[:, None]
    f = np.arange(LF, dtype=np.float64)[None, :]
    ang = 2.0 * np.pi * f * t / L
    rows = 128 * ((L + 127) // 128)
    dc = np.zeros((rows, LF), np.float32)
    ds = np.zeros((rows, LF), np.float32)
    dc[:L] = np.cos(ang)
    ds[:L] = -np.sin(ang)
    return dc, ds


def _idft_mat(L, LF, nfc):
    # rows: chunks 0..nfc-1 = Sre (f), chunks nfc..2nfc-1 = Sim (f); corr scale 1/(L*D)
    t = np.arange(L, dtype=np.float64)[None, :]
    f = np.arange(LF, dtype=np.float64)[:, None]
    ang = 2.0 * np.pi * f * t / L
    w = np.full((LF, 1), 2.0)
    w[0, 0] = 1.0
    w[-1, 0] = 1.0
    scale = 1.0 / (L * D)
    icr = (w * np.cos(ang) * scale).astype(np.float32)
    ism = (-w * np.sin(ang) * scale).astype(np.float32)
    out = np.zeros((2 * nfc * 128, L), np.float32)
    out[:LF] = icr
    out[nfc * 128:nfc * 128 + LF] = ism
    return out


def _chunked_bias(b):
    # [C*128] -> [128, C] per-partition layout
    c = b.shape[0] // 128
    return np.ascontiguousarray(b.reshape(c, 128).T).astype(np.float32)


def host_prep(inp):
    """Return (shared weight/const map, per-core input maps)."""
    g = {}

    def wT(w):  # torch Linear weight [out,in] -> [in,out]
        return np.ascontiguousarray(np.asarray(w).T).astype(np.float32)

    # embeddings: combined [67, 512] (rows j*21+c from tok_W[o,c,j], + time rows)
    for pre, tok, tim in (("e", inp["enc_tok_W"], inp["enc_time_W"]),
                          ("d", inp["dec_tok_W"], inp["dec_time_W"])):
        tok = np.asarray(tok)  # [512, 21, 3]
        m = np.transpose(tok, (2, 1, 0)).reshape(63, D)  # row j*21+c
        t = np.asarray(tim)  # [512, 4]
        g[f"embW_{pre}"] = np.concatenate([m, t.T], 0).astype(np.float32)  # [67,512]

    for l in range(EL):
        for nm in ("q", "k", "v", "o"):
            g[f"eW{nm}{l}"] = wT(inp[f"enc_W{nm}"][l])
            g[f"eb{nm}{l}"] = np.asarray(inp[f"enc_b{nm}"][l]).astype(np.float32)[None, :]
        g[f"eW1{l}"] = wT(inp["enc_W1"][l])
        g[f"eb1{l}"] = _chunked_bias(np.asarray(inp["enc_b1"][l]))
        g[f"eW2{l}"] = wT(inp["enc_W2"][l])
        g[f"eb2{l}"] = _chunked_bias(np.asarray(inp["enc_b2"][l]))
        g[f"ebvC{l}"] = _chunked_bias(np.asarray(inp["enc_bv"][l]))
        g[f"eboC{l}"] = _chunked_bias(np.asarray(inp["enc_bo"][l]))
    for pre in ("ds", "dc"):
        for nm in ("q", "k", "v", "o"):
            g[f"{pre}W{nm}"] = wT(inp[f"{pre}_W{nm}"])
            g[f"{pre}b{nm}"] = np.asarray(inp[f"{pre}_b{nm}"]).astype(np.float32)[None, :]
        g[f"{pre}bvC"] = _chunked_bias(np.asarray(inp[f"{pre}_bv"]))
        g[f"{pre}boC"] = _chunked_bias(np.asarray(inp[f"{pre}_bo"]))
    g["dW1"] = wT(inp["dec_W1"])
    g["db1"] = _chunked_bias(np.asarray(inp["dec_b1"]))
    g["dW2"] = wT(inp["dec_W2"])
    g["db2"] = _chunked_bias(np.asarray(inp["dec_b2"]))

    tw = np.asarray(inp["dec_trend_W"])  # [21, 512, 3]
    g["trendW"] = np.transpose(tw, (2, 1, 0)).reshape(3 * D, CIN).astype(np.float32)
    g["projW"] = wT(inp["dec_proj_W"])  # [512, 21]
    g["projB"] = np.asarray(inp["dec_proj_b"]).astype(np.float32)[:, None]  # [21,1]
    g["enW"] = np.ascontiguousarray(
        np.asarray(inp["enc_norm_w"]).reshape(DC, 128).T).astype(np.float32)
    g["enB"] = np.ascontiguousarray(
        np.asarray(inp["enc_norm_b"]).reshape(DC, 128).T).astype(np.float32)
    g["dnW"] = np.ascontiguousarray(
        np.asarray(inp["dec_norm_w"]).reshape(DC, 128).T).astype(np.float32)
    g["dnB"] = np.ascontiguousarray(
        np.asarray(inp["dec_norm_b"]).reshape(DC, 128).T).astype(np.float32)

    g["ones512"] = np.ones((1, 512), np.float32)
    g["onescol"] = np.ones((128, 1), np.float32)
    g["zeros"] = np.zeros((128, 1056), np.float32)
    g["dftc1"], g["dfts1"] = _dft_mats(L1, LF1)
    g["idft1"] = _idft_mat(L1, LF1, NFC1)
    g["dftc2"], g["dfts2"] = _dft_mats(L2, LF2)
    g["idft2"] = _idft_mat(L2, LF2, NFC2)

    xT = np.transpose(np.asarray(inp["x_enc"]), (0, 2, 1)).astype(np.float32)
    mke = np.transpose(np.asarray(inp["x_mark_enc"]), (0, 2, 1)).astype(np.float32)
    mkd = np.transpose(np.asarray(inp["x_mark_dec"]), (0, 2, 1)).astype(np.float32)
    xT = np.ascontiguousarray(xT)
    mke = np.ascontiguousarray(mke)
    mkd = np.ascontiguousarray(mkd)

    per_core = []
    for c in range(NCORES):
        sl = slice(c * BC, (c + 1) * BC)
        m = dict(g)
        m["xT"] = np.ascontiguousarray(xT[sl])
        m["mkeT"] = np.ascontiguousarray(mke[sl])
        m["mkdT"] = np.ascontiguousarray(mkd[sl])
        per_core.append(m)
    return per_core


# ---------------------------------------------------------------- device build
def build_nc(use_f32r=True, sim=False):
    MMDT = F32R if use_f32r else F32
    GELU = AF.Identity if sim else AF.Gelu
    nc = bacc.Bacc(None, target_bir_lowering=False)

    dram = {}

    def din(name, shape, dt=MMDT):
        dram[name] = nc.dram_tensor(name, list(shape), dt, kind="ExternalInput")
        return dram[name]

    # inputs
    din("xT", (BC, CIN, L1))
    din("mkeT", (BC, 4, L1))
    din("mkdT", (BC, 4, L2))
    din("embW_e", (67, D))
    din("embW_d", (67, D))
    for l in range(EL):
        for nm in ("q", "k", "v", "o"):
            din(f"eW{nm}{l}", (D, D))
            din(f"eb{nm}{l}", (1, D))
        din(f"eW1{l}", (D, DFF))
        din(f"eb1{l}", (128, DFC), F32)
        din(f"eW2{l}", (DFF, D))
        din(f"eb2{l}", (128, DC), F32)
        din(f"ebvC{l}", (128, DC), F32)
        din(f"eboC{l}", (128, DC), F32)
    for pre in ("ds", "dc"):
        for nm in ("q", "k", "v", "o"):
            din(f"{pre}W{nm}", (D, D))
            din(f"{pre}b{nm}", (1, D))
        din(f"{pre}bvC", (128, DC), F32)
        din(f"{pre}boC", (128, DC), F32)
    din("dW1", (D, DFF))
    din("db1", (128, DFC), F32)
    din("dW2", (DFF, D))
    din("db2", (128, DC), F32)
    din("trendW", (3 * D, CIN))
    din("projW", (D, CIN))
    din("projB", (CIN, 1), F32)
    for nm in ("enW", "enB", "dnW", "dnB"):
        din(nm, (128, DC), F32)
    din("ones512", (1, 512))
    din("onescol", (128, 1))
    din("zeros", (128, 1056))
    din("dftc1", (TC1 * 128, LF1))
    din("dfts1", (TC1 * 128, LF1))
    din("idft1", (2 * NFC1 * 128, L1))
    din("dftc2", (TC2 * 128, LF2))
    din("dfts2", (TC2 * 128, LF2))
    din("idft2", (2 * NFC2 * 128, L2))

    out_d = nc.dram_tensor("out", [BC, CIN, PRED], F32, kind="ExternalOutput")

    # DRAM scratch
    enc_dram = nc.dram_tensor("enc_scratch", [BC, DC, 128, L1], MMDT)
    dec_dram = nc.dram_tensor("dec_scratch", [BC, DC, 128, L2], MMDT)
    tsum_dram = nc.dram_tensor("tsum_scratch", [BC, DC, 128, L2], F32)
    seas_dram = nc.dram_tensor("seas_scratch", [BC, CIN, LAB], MMDT)
    trendi_dram = nc.dram_tensor("trendi_scratch", [BC, CIN, L2], F32)

    with TileContext(nc) as tc, ExitStack() as top:
        cpool = top.enter_context(tc.tile_pool(name="consts", bufs=1))
        ones_row = cpool.tile([1, 512], MMDT)
        nc.sync.dma_start(ones_row, dram["ones512"][:])
        ones_col = cpool.tile([128, 1], MMDT)
        nc.sync.dma_start(ones_col, dram["onescol"][:])
        zeros_t = cpool.tile([128, 1056], MMDT)
        nc.sync.dma_start(zeros_t, dram["zeros"][:])

        # ---------------------------------------------------- helper closures
        def linear_T(ps_pool, out, X, W, bias, L, tcn):
            """out[128, tcn, 512] (seq-part) = X.T @ W + bias ; X[128,DC,L]."""
            if L % 128:
                nc.vector.tensor_copy(out[:, tcn - 1, :], zeros_t[:, 0:512])
            for mt in range(tcn):
                m = min(128, L - mt * 128)
                ps = ps_pool.tile([128, 512], F32, tag="mm")
                for kc in range(DC):
                    nc.tensor.matmul(ps[0:m, :], X[:, kc, mt * 128:mt * 128 + m],
                                     W[:, kc, :], start=(kc == 0), stop=False)
                nc.tensor.matmul(ps[0:m, :], ones_row[0:1, 0:m], bias,
                                 start=False, stop=True)
                nc.scalar.copy(out[0:m, mt, :], ps[0:m, :])

        def dft_S(ps_pool, tmp_pool, Sstk, q, k, dftc, dfts, fts, nfc, tck, tcq):
            """Sstk[128, 2*nfc, 1] f32r: stacked sum_c Qf*conj(Kf)."""
            nc.vector.tensor_copy(Sstk[:, :, 0], zeros_t[:, 0:Sstk.shape[1]])
            for ft, (f0, fm) in enumerate(fts):
                scr = {}
                for nm, mat, src, tcs in (("qr", dftc, q, tcq), ("qi", dfts, q, tcq),
                                          ("kr", dftc, k, tck), ("ki", dfts, k, tck)):
                    ps = ps_pool.tile([128, 512], F32, tag="pdft", bufs=2,
                                      name=f"pdft_{nm}")
                    for t in range(tcs):
                        nc.tensor.matmul(ps[0:fm, :], mat[:, t, f0:f0 + fm],
                                         src[:, t, :], start=(t == 0),
                                         stop=(t == tcs - 1))
                    sc = tmp_pool.tile([128, 512], F32, tag=f"s{nm}",
                                       name=f"s{nm}")
                    nc.scalar.copy(sc[0:fm, :], ps[0:fm, :])
                    scr[nm] = sc
                prod = tmp_pool.tile([128, 512], F32, tag="prod", bufs=2)
                cols = tmp_pool.tile([128, 4], F32, tag="cols", bufs=2)
                for ci, (xa, xb) in enumerate((("qr", "kr"), ("qi", "ki"),
                                               ("qi", "kr"), ("qr", "ki"))):
                    nc.vector.scalar_tensor_tensor(
                        prod[0:fm, :], scr[xa][0:fm, :], 1.0, scr[xb][0:fm, :],
                        op0=AL.bypass, op1=AL.mult,
                        accum_out=cols[0:fm, ci:ci + 1])
                nc.vector.tensor_tensor(Sstk[0:fm, ft, 0:1], cols[0:fm, 0:1],
                                        cols[0:fm, 1:2], AL.add)
                nc.vector.tensor_tensor(Sstk[0:fm, nfc + ft, 0:1], cols[0:fm, 2:3],
                                        cols[0:fm, 3:4], AL.subtract)

        def topk_tw(ps_pool, tmp_pool, Sstk, idft, nfc, L, nts):
            """corr -> (twb[128,8] f32, i8[1,8] u32)."""
            corr = tmp_pool.tile([1, L], F32, tag="corr")
            for nt, (n0, n) in enumerate(nts):
                psc = ps_pool.tile([1, 512], F32, tag="corrps")
                for j in range(2 * nfc):
                    nc.tensor.matmul(psc[:, 0:n], Sstk[:, j, 0:1],
                                     idft[:, j, n0:n0 + n],
                                     start=(j == 0), stop=(j == 2 * nfc - 1))
                nc.scalar.copy(corr[:, n0:n0 + n], psc[:, 0:n])
            w8 = tmp_pool.tile([1, 8], F32, tag="w8")
            i8 = tmp_pool.tile([1, 8], U32, tag="i8")
            nc.vector.max_with_indices(w8, i8, corr)
            e6 = tmp_pool.tile([1, 8], F32, tag="e6")
            nc.vector.memset(e6[:, TOPK:8], 0.0)
            nc.vector.tensor_scalar_sub(e6[:, 0:TOPK], w8[:, 0:TOPK], w8[:, 0:1])
            nc.scalar.activation(e6[:, 0:TOPK], e6[:, 0:TOPK], AF.Exp)
            ssum = tmp_pool.tile([1, 1], F32, tag="ssum")
            nc.vector.reduce_sum(ssum, e6[:, 0:TOPK], axis=AX.X)
            nc.vector.reciprocal(ssum, ssum)
            nc.vector.tensor_scalar_mul(e6[:, 0:TOPK], e6[:, 0:TOPK], ssum)
            twb = tmp_pool.tile([128, 8], F32, tag="twb")
            nc.gpsimd.partition_broadcast(twb, e6[0:1, :])
            return twb, i8

        def agg_delays(agg, vv, twb, i8, L):
            for kk in range(TOPK):
                dly = nc.values_load(i8[0:1, kk:kk + 1], min_val=0, max_val=L - 1,
                                     engines=[DVE], skip_runtime_bounds_check=True)
                for mc in range(DC):
                    src = vv[:, mc, bass.ds(dly, L)]
                    if kk == 0:
                        nc.vector.scalar_tensor_tensor(
                            agg[:, mc, :], src, twb[:, 0:1], src,
                            op0=AL.mult, op1=AL.bypass)
                    else:
                        nc.vector.scalar_tensor_tensor(
                            agg[:, mc, :], src, twb[:, kk:kk + 1], agg[:, mc, :],
                            op0=AL.mult, op1=AL.add)

        def out_proj_residual(ps_pool, X, agg, W, boC, nts):
            """X += agg.T@W + bo  (F-layout, in place)."""
            for mc in range(DC):
                for (n0, n) in nts:
                    ps = ps_pool.tile([128, 512], F32, tag="mm")
                    for kc in range(DC):
                        nc.tensor.matmul(ps[:, 0:n], W[:, kc, mc * 128:(mc + 1) * 128],
                                         agg[:, kc, n0:n0 + n],
                                         start=(kc == 0), stop=(kc == DC - 1))
                    nc.vector.scalar_tensor_tensor(
                        X[:, mc, n0:n0 + n], ps[:, 0:n], boC[:, mc:mc + 1],
                        X[:, mc, n0:n0 + n], op0=AL.add, op1=AL.add)

        def decomp(tmp_pool, X, L, chunks=DC, trend_to=None, trend_accum=None,
                   trend_dram_b=None, db=1):
            """X <- X - mavg(X) in place; optionally emit trend (mavg)."""
            for mc in range(chunks):
                xp = tmp_pool.tile([128, L + 2 * PAD], F32, tag="xp", bufs=db)
                nc.scalar.copy(xp[:, PAD:PAD + L], X[:, mc, :])
                nc.vector.tensor_copy(xp[:, 0:PAD],
                                      X[:, mc, 0:1].to_broadcast([128, PAD]))
                nc.vector.tensor_copy(xp[:, PAD + L:],
                                      X[:, mc, L - 1:L].to_broadcast([128, PAD]))
                cs = tmp_pool.tile([128, L + 2 * PAD + 1], F32, tag="cs", bufs=db)
                nc.vector.memset(cs[:, 0:1], 0.0)
                nc.vector.tensor_tensor_scan(cs[:, 1:], xp, xp, 0.0, AL.add, AL.bypass)
                dt = tmp_pool.tile([128, L], F32, tag="dt", bufs=2)
                nc.vector.tensor_tensor(dt, cs[:, MA:MA + L], cs[:, 0:L], AL.subtract)
                if trend_to is not None:
                    nc.vector.tensor_scalar_mul(trend_to[:, mc, :], dt, 1.0 / MA)
                if trend_accum is not None:
                    tt = tmp_pool.tile([128, L], F32, tag="taccum", bufs=2)
                    nc.vector.tensor_scalar_mul(tt, dt, 1.0 / MA)
                    nc.gpsimd.dma_start(tsum_dram[trend_dram_b, mc], tt,
                                        accum_op=(AL.add if trend_accum == "add"
                                                  else AL.bypass))
                nc.vector.scalar_tensor_tensor(X[:, mc, :], dt, -1.0 / MA,
                                               X[:, mc, :], op0=AL.mult, op1=AL.add)

        def ffn(ps_pool, tmp_pool, X, W1, b1C, W2, b2C, L, nts):
            """X += gelu(X@W1+b1)@W2+b2 in place (F-layout)."""
            h = tmp_pool.tile([128, DFC, L], MMDT, tag="h")
            for mh in range(DFC):
                for (n0, n) in nts:
                    ps = ps_pool.tile([128, 512], F32, tag="mm")
                    for kc in range(DC):
                        nc.tensor.matmul(ps[:, 0:n], W1[:, kc, mh * 128:(mh + 1) * 128],
                                         X[:, kc, n0:n0 + n],
                                         start=(kc == 0), stop=(kc == DC - 1))
                    nc.scalar.activation(h[:, mh, n0:n0 + n], ps[:, 0:n], GELU,
                                         bias=b1C[:, mh:mh + 1])
            for mc in range(DC):
                for (n0, n) in nts:
                    ps = ps_pool.tile([128, 512], F32, tag="mm")
                    for kh in range(DFC):
                        nc.tensor.matmul(ps[:, 0:n], W2[:, kh, mc * 128:(mc + 1) * 128],
                                         h[:, kh, n0:n0 + n],
                                         start=(kh == 0), stop=(kh == DFC - 1))
                    nc.vector.scalar_tensor_tensor(
                        X[:, mc, n0:n0 + n], ps[:, 0:n], b2C[:, mc:mc + 1],
                        X[:, mc, n0:n0 + n], op0=AL.add, op1=AL.add)

        def layernorm(ps_pool, tmp_pool, X, Xln, L, nts, wD, bD):
            """Xln = LN(X) over feature dim (partition dim, DC chunks)."""
            xsq = tmp_pool.tile([128, DC, L], MMDT, tag="xsq")
            for mc in range(DC):
                nc.scalar.activation(xsq[:, mc, :], X[:, mc, :], AF.Square)
            pmu = ps_pool.tile([1, len(nts), 512], F32, tag="pmu")
            psq = ps_pool.tile([1, len(nts), 512], F32, tag="psq")
            for nt, (n0, n) in enumerate(nts):
                for kc in range(DC):
                    st, sp = (kc == 0), (kc == DC - 1)
                    nc.tensor.matmul(pmu[:, nt, 0:n], ones_col, X[:, kc, n0:n0 + n],
                                     start=st, stop=sp)
                    nc.tensor.matmul(psq[:, nt, 0:n], ones_col, xsq[:, kc, n0:n0 + n],
                                     start=st, stop=sp)
            stats = tmp_pool.tile([1, 2 * L], F32, tag="stats")
            mu, rstd = stats[:, 0:L], stats[:, L:2 * L]
            for nt, (n0, n) in enumerate(nts):
                nc.vector.tensor_scalar_mul(mu[:, n0:n0 + n], pmu[:, nt, 0:n], 1.0 / D)
                nc.vector.tensor_scalar_mul(rstd[:, n0:n0 + n], psq[:, nt, 0:n], 1.0 / D)
            musq = tmp_pool.tile([1, L], F32, tag="musq")
            nc.vector.tensor_tensor(musq, mu, mu, AL.mult)
            nc.vector.tensor_tensor(rstd, rstd, musq, AL.subtract)
            nc.vector.tensor_scalar_add(rstd, rstd, 1e-5)
            nc.scalar.activation(rstd, rstd, AF.Sqrt)
            nc.vector.reciprocal(rstd, rstd)
            stb = tmp_pool.tile([128, 2 * L], F32, tag="stb")
            nc.gpsimd.partition_broadcast(stb, stats[0:1, :])
            t = tmp_pool.tile([128, L], F32, tag="lnt")
            for mc in range(DC):
                nc.vector.tensor_tensor(t, X[:, mc, :], stb[:, 0:L], AL.subtract)
                nc.vector.tensor_tensor(t, t, stb[:, L:2 * L], AL.mult)
                nc.vector.tensor_scalar_mul(t, t, wD[:, mc:mc + 1])
                nc.vector.tensor_scalar_add(Xln[:, mc, :], t, bD[:, mc:mc + 1])

        # ======================================================== ENCODER
        with tc.tile_pool(name="acts", bufs=1) as apool:
            enc_acts = [apool.tile([128, DC, L1], MMDT, tag=f"enc{b}",
                                   name=f"enc_acts{b}")
                        for b in range(BC)]

            # ---- P0: embedding + init decomposition
            with nc.named_scope("P0_embed"), \
                 tc.tile_pool(name="p0t", bufs=2) as tp, \
                 tc.tile_pool(name="p0w", bufs=1) as wp, \
                 tc.tile_pool(name="p0ps", bufs=4, space="PSUM") as psp:
                embW = wp.tile([67, D], MMDT)
                nc.sync.dma_start(embW, dram["embW_e"][:])
                for b in range(BC):
                    win = tp.tile([67, L1], MMDT, tag="win")
                    nc.sync.dma_start(win[0:CIN, 1:L1], dram["xT"][b, :, 0:L1 - 1])
                    nc.sync.dma_start(win[0:CIN, 0:1], dram["xT"][b, :, L1 - 1:L1])
                    nc.sync.dma_start(win[CIN:2 * CIN, :], dram["xT"][b])
                    nc.sync.dma_start(win[2 * CIN:3 * CIN, 0:L1 - 1],
                                      dram["xT"][b, :, 1:L1])
                    nc.sync.dma_start(win[2 * CIN:3 * CIN, L1 - 1:L1],
                                      dram["xT"][b, :, 0:1])
                    nc.sync.dma_start(win[63:67, :], dram["mkeT"][b])
                    for mc in range(DC):
                        for (n0, n) in NT1:
                            ps = psp.tile([128, 512], F32, tag="mm")
                            nc.tensor.matmul(ps[:, 0:n],
                                             embW[:, mc * 128:(mc + 1) * 128],
                                             win[:, n0:n0 + n], start=True, stop=True)
                            nc.scalar.copy(enc_acts[b][:, mc, n0:n0 + n], ps[:, 0:n])

                # init series_decomp of x_enc (packed [84, .])
                xe = tp.tile([84, L1], MMDT, tag="xe")
                for b in range(BC):
                    nc.sync.dma_start(xe[b * CIN:(b + 1) * CIN, :], dram["xT"][b])
                xp = tp.tile([84, L1 + 2 * PAD], F32, tag="ixp")
                nc.scalar.copy(xp[:, PAD:PAD + L1], xe)
                nc.vector.tensor_copy(xp[:, 0:PAD], xe[:, 0:1].to_broadcast([84, PAD]))
                nc.vector.tensor_copy(xp[:, PAD + L1:],
                                      xe[:, L1 - 1:L1].to_broadcast([84, PAD]))
                cs = tp.tile([84, L1 + 2 * PAD + 1], F32, tag="ics")
                nc.vector.memset(cs[:, 0:1], 0.0)
                nc.vector.tensor_tensor_scan(cs[:, 1:], xp, xp, 0.0, AL.add, AL.bypass)
                dt = tp.tile([84, L1], F32, tag="idt")
                nc.vector.tensor_tensor(dt, cs[:, MA:MA + L1], cs[:, 0:L1], AL.subtract)
                seas = tp.tile([84, L1], MMDT, tag="iseas")
                nc.vector.scalar_tensor_tensor(seas, dt, -1.0 / MA, xe,
                                               op0=AL.mult, op1=AL.add)
                trend = tp.tile([84, L1], F32, tag="itrend")
                nc.vector.tensor_scalar_mul(trend, dt, 1.0 / MA)
                mean = tp.tile([84, 1], F32, tag="imean")
                nc.vector.reduce_sum(mean, xe, axis=AX.X)
                nc.vector.tensor_scalar_mul(mean, mean, 1.0 / L1)
                meanb = tp.tile([84, PRED], F32, tag="imeanb")
                nc.vector.tensor_copy(meanb, mean.to_broadcast([84, PRED]))
                for b in range(BC):
                    sl = slice(b * CIN, (b + 1) * CIN)
                    nc.sync.dma_start(seas_dram[b], seas[sl, L1 - LAB:L1])
                    nc.sync.dma_start(trendi_dram[b, :, 0:LAB], trend[sl, L1 - LAB:L1])
                    nc.sync.dma_start(trendi_dram[b, :, LAB:L2], meanb[sl, :])

            # ---- P1/P2: encoder layers
            if True:
                for l in range(EL):
                    with nc.named_scope(f"enc{l}_att"), \
                         tc.tile_pool(name="dft1", bufs=1) as dft1p, \
                         tc.tile_pool(name="eatw", bufs=1) as wp, \
                         tc.tile_pool(name="eatt", bufs=1) as tp, \
                         tc.tile_pool(name="eatps", bufs=1, space="PSUM") as psp, \
                         tc.tile_pool(name="eatps2", bufs=3, space="PSUM") as psp2:
                        dftc1 = dft1p.tile([128, TC1, LF1], MMDT, tag="dftc1")
                        dfts1 = dft1p.tile([128, TC1, LF1], MMDT, tag="dfts1")
                        idft1 = dft1p.tile([128, 2 * NFC1, L1], MMDT, tag="idft1")
                        nc.sync.dma_start(dftc1, dram["dftc1"][:].rearrange(
                            "(c p) f -> p c f", p=128))
                        nc.sync.dma_start(dfts1, dram["dfts1"][:].rearrange(
                            "(c p) f -> p c f", p=128))
                        nc.sync.dma_start(idft1, dram["idft1"][:].rearrange(
                            "(c p) f -> p c f", p=128))
                        Ws = {}
                        for nm in ("q", "k", "v", "o"):
                            Ws[nm] = wp.tile([128, DC, D], MMDT, tag=f"W{nm}",
                                             name=f"W{nm}")
                            nc.sync.dma_start(Ws[nm], dram[f"eW{nm}{l}"][:].rearrange(
                                "(c p) f -> p c f", p=128))
                        bq = wp.tile([1, D], MMDT, tag="bq")
                        bk = wp.tile([1, D], MMDT, tag="bk")
                        nc.sync.dma_start(bq, dram[f"ebq{l}"][:])
                        nc.sync.dma_start(bk, dram[f"ebk{l}"][:])
                        bvC = wp.tile([128, DC], F32, tag="bvC")
                        boC = wp.tile([128, DC], F32, tag="boC")
                        nc.sync.dma_start(bvC, dram[f"ebvC{l}"][:])
                        nc.sync.dma_start(boC, dram[f"eboC{l}"][:])
                        for b in range(BC):
                            X = enc_acts[b]
                            q = tp.tile([128, TC1, 512], MMDT, tag="q")
                            k = tp.tile([128, TC1, 512], MMDT, tag="k", bufs=2)
                            linear_T(psp2, q, X, Ws["q"], bq, L1, TC1)
                            linear_T(psp2, k, X, Ws["k"], bk, L1, TC1)
                            Sstk = tp.tile([128, 2 * NFC1, 1], MMDT, tag="Sstk")
                            dft_S(psp, tp, Sstk, q, k, dftc1, dfts1, FT1, NFC1,
                                  TC1, TC1)
                            twb, i8 = topk_tw(psp, tp, Sstk, idft1, NFC1, L1, NT1)
                            vv = tp.tile([128, DC, 2 * L1], F32, tag="q")
                            for mc in range(DC):
                                for (n0, n) in NT1:
                                    ps = psp2.tile([128, 512], F32, tag="mm")
                                    for kc in range(DC):
                                        nc.tensor.matmul(
                                            ps[:, 0:n],
                                            Ws["v"][:, kc, mc * 128:(mc + 1) * 128],
                                            X[:, kc, n0:n0 + n],
                                            start=(kc == 0), stop=(kc == DC - 1))
                                    nc.scalar.activation(vv[:, mc, n0:n0 + n],
                                                         ps[:, 0:n], AF.Identity,
                                                         bias=bvC[:, mc:mc + 1])
                                    nc.scalar.activation(vv[:, mc, L1 + n0:L1 + n0 + n],
                                                         ps[:, 0:n], AF.Identity,
                                                         bias=bvC[:, mc:mc + 1])
                            agg = tp.tile([128, DC, L1], MMDT, tag="k", bufs=2)
                            agg_delays(agg, vv, twb, i8, L1)
                            out_proj_residual(psp2, X, agg, Ws["o"], boC, NT1)
                            decomp(tp, X, L1)

                    with nc.named_scope(f"enc{l}_ffn"), \
                         tc.tile_pool(name="effw", bufs=1) as wp, \
                         tc.tile_pool(name="efft", bufs=1) as tp, \
                         tc.tile_pool(name="effps", bufs=6, space="PSUM") as psp:
                        W1 = wp.tile([128, DC, DFF], MMDT, tag="W1")
                        W2 = wp.tile([128, DFC, D], MMDT, tag="W2")
                        nc.sync.dma_start(W1, dram[f"eW1{l}"][:].rearrange(
                            "(c p) f -> p c f", p=128))
                        nc.sync.dma_start(W2, dram[f"eW2{l}"][:].rearrange(
                            "(c p) f -> p c f", p=128))
                        b1C = wp.tile([128, DFC], F32, tag="b1C")
                        b2C = wp.tile([128, DC], F32, tag="b2C")
                        nc.sync.dma_start(b1C, dram[f"eb1{l}"][:])
                        nc.sync.dma_start(b2C, dram[f"eb2{l}"][:])
                        for b in range(BC):
                            ffn(psp, tp, enc_acts[b], W1, b1C, W2, b2C, L1, NT1)
                            decomp(tp, enc_acts[b], L1, db=2)

            # ---- P3: final encoder LN -> enc_dram
            with nc.named_scope("enc_ln"), \
                 tc.tile_pool(name="lnt", bufs=1) as tp, \
                 tc.tile_pool(name="lnw", bufs=1) as wp, \
                 tc.tile_pool(name="lnps", bufs=1, space="PSUM") as psp:
                enW = wp.tile([128, DC], F32, tag="enW")
                enB = wp.tile([128, DC], F32, tag="enB")
                nc.sync.dma_start(enW, dram["enW"][:])
                nc.sync.dma_start(enB, dram["enB"][:])
                for b in range(BC):
                    xln = tp.tile([128, DC, L1], MMDT, tag="xln")
                    layernorm(psp, tp, enc_acts[b], xln, L1, NT1, enW, enB)
                    for mc in range(DC):
                        nc.sync.dma_start(enc_dram[b, mc], xln[:, mc, :])

        # ======================================================== DECODER
        # ---- P4: decoder embedding -> dec_dram
        with nc.named_scope("dec_embed"), \
             tc.tile_pool(name="p4t", bufs=2) as tp, \
             tc.tile_pool(name="p4w", bufs=1) as wp, \
             tc.tile_pool(name="p4ps", bufs=4, space="PSUM") as psp:
            embW = wp.tile([67, D], MMDT)
            nc.sync.dma_start(embW, dram["embW_d"][:])
            for b in range(BC):
                win = tp.tile([67, L2], MMDT, tag="win2")
                nc.vector.tensor_copy(win[0:63, :], zeros_t[0:63, 0:L2])
                nc.sync.dma_start(win[0:CIN, 1:LAB + 1], seas_dram[b])
                nc.sync.dma_start(win[CIN:2 * CIN, 0:LAB], seas_dram[b])
                nc.sync.dma_start(win[2 * CIN:3 * CIN, 0:LAB - 1],
                                  seas_dram[b, :, 1:LAB])
                nc.sync.dma_start(win[2 * CIN:3 * CIN, L2 - 1:L2],
                                  seas_dram[b, :, 0:1])
                nc.sync.dma_start(win[63:67, :], dram["mkdT"][b])
                for mc in range(DC):
                    for (n0, n) in NT2:
                        ps = psp.tile([128, 512], F32, tag="mm")
                        nc.tensor.matmul(ps[:, 0:n], embW[:, mc * 128:(mc + 1) * 128],
                                         win[:, n0:n0 + n], start=True, stop=True)
                        xpart = tp.tile([128, 512], MMDT, tag="xpart")
                        nc.scalar.copy(xpart[:, 0:n], ps[:, 0:n])
                        nc.sync.dma_start(dec_dram[b, mc, :, n0:n0 + n], xpart[:, 0:n])

        # ---- P5/P6: decoder attentions
        for phase, pre in (("self", "ds"), ("cross", "dc")):
            with nc.named_scope(f"dec_{phase}"), \
                 tc.tile_pool(name="datw", bufs=1) as wp, \
                 tc.tile_pool(name="datt", bufs=1) as tp, \
                 tc.tile_pool(name="dft2", bufs=1) as dp, \
                 tc.tile_pool(name="datps", bufs=1, space="PSUM") as psp, \
                 tc.tile_pool(name="datps2", bufs=3, space="PSUM") as psp2:
                Ws = {}
                for nm in ("q", "k", "v", "o"):
                    Ws[nm] = wp.tile([128, DC, D], MMDT, tag=f"W{nm}",
                                     name=f"W{nm}")
                    nc.sync.dma_start(Ws[nm], dram[f"{pre}W{nm}"][:].rearrange(
                        "(c p) f -> p c f", p=128))
                bq = wp.tile([1, D], MMDT, tag="bq")
                bk = wp.tile([1, D], MMDT, tag="bk")
                nc.sync.dma_start(bq, dram[f"{pre}bq"][:])
                nc.sync.dma_start(bk, dram[f"{pre}bk"][:])
                bvC = wp.tile([128, DC], F32, tag="bvC")
                boC = wp.tile([128, DC], F32, tag="boC")
                nc.sync.dma_start(bvC, dram[f"{pre}bvC"][:])
                nc.sync.dma_start(boC, dram[f"{pre}boC"][:])
                kvsrc_chunks = TC2 if phase == "self" else TC1
                for b in range(BC):
                    X = tp.tile([128, DC, L2], MMDT, tag="Xd")
                    for mc in range(DC):
                        nc.sync.dma_start(X[:, mc, :], dec_dram[b, mc])
                    if phase == "self":
                        KV = X
                    else:
                        KV = tp.tile([128, DC, L1], MMDT, tag="KV")
                        for mc in range(DC):
                            nc.sync.dma_start(KV[:, mc, :], enc_dram[b, mc])
                    q = tp.tile([128, TC2, 512], MMDT, tag="q2")
                    k = tp.tile([128, TC2, 512], MMDT, tag="k2")
                    linear_T(psp2, q, X, Ws["q"], bq, L2, TC2)
                    linear_T(psp2, k, KV, Ws["k"], bk,
                             L2 if phase == "self" else L1, kvsrc_chunks)
                    dftcs = dp.tile([128, TC2, 2 * LF2], MMDT, tag="dftbig")
                    nc.sync.dma_start(dftcs[:, :, 0:LF2], dram["dftc2"][:].rearrange(
                        "(c p) f -> p c f", p=128))
                    nc.sync.dma_start(dftcs[:, :, LF2:], dram["dfts2"][:].rearrange(
                        "(c p) f -> p c f", p=128))
                    Sstk = tp.tile([128, 2 * NFC2, 1], MMDT, tag="Sstk2")
                    dft_S(psp, tp, Sstk, q, k, dftcs[:, :, 0:LF2],
                          dftcs[:, :, LF2:2 * LF2], FT2, NFC2, kvsrc_chunks, TC2)
                    idft2 = dp.tile([128, 2 * NFC2, L2], MMDT, tag="dftbig")
                    nc.sync.dma_start(idft2, dram["idft2"][:].rearrange(
                        "(c p) f -> p c f", p=128))
                    twb, i8 = topk_tw(psp, tp, Sstk, idft2, NFC2, L2, NT2)
                    VL = 2 * L2
                    vv = tp.tile([128, DC, VL], F32, tag="q2")
                    if phase == "cross":
                        nc.vector.memset(vv, 0.0)
                    kvL = L2 if phase == "self" else L1
                    kvNT = NT2 if phase == "self" else NT1
                    for mc in range(DC):
                        for (n0, n) in kvNT:
                            ps = psp2.tile([128, 512], F32, tag="mm")
                            for kc in range(DC):
                                nc.tensor.matmul(
                                    ps[:, 0:n],
                                    Ws["v"][:, kc, mc * 128:(mc + 1) * 128],
                                    KV[:, kc, n0:n0 + n],
                                    start=(kc == 0), stop=(kc == DC - 1))
                            nc.scalar.activation(vv[:, mc, n0:n0 + n], ps[:, 0:n],
                                                 AF.Identity, bias=bvC[:, mc:mc + 1])
                            nc.scalar.activation(vv[:, mc, L2 + n0:L2 + n0 + n],
                                                 ps[:, 0:n], AF.Identity,
                                                 bias=bvC[:, mc:mc + 1])
                    agg = tp.tile([128, DC, L2], MMDT, tag="k2")
                    agg_delays(agg, vv, twb, i8, L2)
                    out_proj_residual(psp2, X, agg, Ws["o"], boC, NT2)
                    decomp(tp, X, L2, trend_accum=("bypass" if phase == "self"
                                                   else "add"), trend_dram_b=b)
                    for mc in range(DC):
                        nc.sync.dma_start(dec_dram[b, mc], X[:, mc, :])

        # ---- P7: decoder FFN
        with nc.named_scope("dec_ffn"), \
             tc.tile_pool(name="dffw", bufs=1) as wp, \
             tc.tile_pool(name="dfft", bufs=1) as tp, \
             tc.tile_pool(name="dffps", bufs=6, space="PSUM") as psp:
            W1 = wp.tile([128, DC, DFF], MMDT, tag="W1")
            W2 = wp.tile([128, DFC, D], MMDT, tag="W2")
            nc.sync.dma_start(W1, dram["dW1"][:].rearrange("(c p) f -> p c f", p=128))
            nc.sync.dma_start(W2, dram["dW2"][:].rearrange("(c p) f -> p c f", p=128))
            b1C = wp.tile([128, DFC], F32, tag="b1C")
            b2C = wp.tile([128, DC], F32, tag="b2C")
            nc.sync.dma_start(b1C, dram["db1"][:])
            nc.sync.dma_start(b2C, dram["db2"][:])
            for b in range(BC):
                X = tp.tile([128, DC, L2], MMDT, tag="Xd")
                for mc in range(DC):
                    nc.sync.dma_start(X[:, mc, :], dec_dram[b, mc])
                ffn(psp, tp, X, W1, b1C, W2, b2C, L2, NT2)
                decomp(tp, X, L2, trend_accum="add", trend_dram_b=b, db=2)
                for mc in range(DC):
                    nc.sync.dma_start(dec_dram[b, mc], X[:, mc, :])

        # ---- P8: final (LN + proj, trend conv, combine)
        with nc.named_scope("final"), \
             tc.tile_pool(name="fint", bufs=1) as tp, \
             tc.tile_pool(name="finw", bufs=1) as wp, \
             tc.tile_pool(name="finps", bufs=1, space="PSUM") as psp, \
             tc.tile_pool(name="finps2", bufs=2, space="PSUM") as psp2:
            trendW = wp.tile([128, 3 * DC, CIN], MMDT, tag="trendW")
            nc.sync.dma_start(trendW, dram["trendW"][:].rearrange(
                "(c p) f -> p c f", p=128))
            projW = wp.tile([128, DC, CIN], MMDT, tag="projW")
            nc.sync.dma_start(projW, dram["projW"][:].rearrange(
                "(c p) f -> p c f", p=128))
            projB = wp.tile([CIN, 1], F32, tag="projB")
            nc.sync.dma_start(projB, dram["projB"][:])
            dnW = wp.tile([128, DC], F32, tag="dnW")
            dnB = wp.tile([128, DC], F32, tag="dnB")
            nc.sync.dma_start(dnW, dram["dnW"][:])
            nc.sync.dma_start(dnB, dram["dnB"][:])
            for b in range(BC):
                X = tp.tile([128, DC, L2], MMDT, tag="Xd")
                for mc in range(DC):
                    nc.sync.dma_start(X[:, mc, :], dec_dram[b, mc])
                xln = tp.tile([128, DC, L2], MMDT, tag="xln2")
                layernorm(psp, tp, X, xln, L2, NT2, dnW, dnB)
                seasonal = tp.tile([CIN, L2], F32, tag="seasonal")
                for (n0, n) in NT2:
                    ps = psp2.tile([CIN, 512], F32, tag="sm")
                    for kc in range(DC):
                        nc.tensor.matmul(ps[:, 0:n], projW[:, kc, :],
                                         xln[:, kc, n0:n0 + n],
                                         start=(kc == 0), stop=(kc == DC - 1))
                    nc.scalar.activation(seasonal[:, n0:n0 + n], ps[:, 0:n],
                                         AF.Identity, bias=projB)
                # trend conv windows [12 chunks, L2] circular
                tsum = tp.tile([128, DC, L2], F32, tag="tsum")
                for mc in range(DC):
                    nc.sync.dma_start(tsum[:, mc, :], tsum_dram[b, mc])
                winT = tp.tile([128, 3 * DC, L2], MMDT, tag="winT")
                for mc in range(DC):
                    # j=0: shift -1 ; j=1: center ; j=2: shift +1 (circular)
                    nc.scalar.copy(winT[:, mc, 1:L2], tsum[:, mc, 0:L2 - 1])
                    nc.scalar.copy(winT[:, mc, 0:1], tsum[:, mc, L2 - 1:L2])
                    nc.scalar.copy(winT[:, DC + mc, :], tsum[:, mc, :])
                    nc.scalar.copy(winT[:, 2 * DC + mc, 0:L2 - 1], tsum[:, mc, 1:L2])
                    nc.scalar.copy(winT[:, 2 * DC + mc, L2 - 1:L2], tsum[:, mc, 0:1])
                trendi = tp.tile([CIN, L2], F32, tag="trendi")
                nc.sync.dma_start(trendi, trendi_dram[b])
                outt = tp.tile([CIN, PRED], F32, tag="outt")
                for nt, (n0, n) in enumerate(NT2):
                    ps = psp2.tile([CIN, 512], F32, tag="sm")
                    for j in range(3 * DC):
                        nc.tensor.matmul(ps[:, 0:n], trendW[:, j, :],
                                         winT[:, j, n0:n0 + n],
                                         start=(j == 0), stop=(j == 3 * DC - 1))
                    trend_sl = tp.tile([CIN, 512], F32, tag="trend_sl")
                    nc.vector.tensor_tensor(trend_sl[:, 0:n], ps[:, 0:n],
                                            trendi[:, n0:n0 + n], AL.add)
                    # add seasonal, write PRED slice (cols >= LAB)
                    lo = max(n0, LAB)
                    hi = n0 + n
                    if hi > lo:
                        nc.vector.tensor_tensor(
                            outt[:, lo - LAB:hi - LAB], trend_sl[:, lo - n0:hi - n0],
                            seasonal[:, lo:hi], AL.add)
                nc.sync.dma_start(out_d[b], outt)

    nc.compile()
    return nc


# ---------------------------------------------------------------- entry point
_CACHE = {}
LAST_EXEC_NS = [None]
SHARDED_INPUTS = ("xT", "mkeT", "mkdT")


def _get_executor(use_f32r=True):
    if "run" in _CACHE:
        return _CACHE["run"]
    import jax
    from jax.sharding import Mesh, PartitionSpec
    try:
        from jax.experimental.shard_map import shard_map
    except Exception:
        from jax.shard_map import shard_map
    from concourse import bass2jax

    bass2jax.install_neuronx_cc_hook()
    nc = build_nc(use_f32r=use_f32r)

    part_name = (nc.partition_id_tensor.name
                 if nc.partition_id_tensor else None)
    in_names, out_names, out_avals = [], [], []
    for alloc in nc.m.functions[0].allocations:
        if not isinstance(alloc, mybir.MemoryLocationSet):
            continue
        name = alloc.memorylocations[0].name
        if alloc.kind == "ExternalInput":
            if name != part_name:
                in_names.append(name)
        elif alloc.kind == "ExternalOutput":
            out_names.append(name)
            out_avals.append(jax.core.ShapedArray(
                tuple(alloc.tensor_shape), mybir.dt.np(alloc.dtype)))
    n_params = len(in_names)
    n_outs = len(out_names)
    all_names = list(in_names) + list(out_names)
    if part_name is not None:
        all_names.append(part_name)

    donate = tuple(range(n_params, n_params + n_outs))

    def _body(*args):
        operands = list(args)
        if part_name is not None:
            operands.append(bass2jax.partition_id_tensor())
        outs = bass2jax._bass_exec_p.bind(
            *operands,
            out_avals=tuple(out_avals),
            in_names=tuple(all_names),
            out_names=tuple(out_names),
            lowering_input_output_aliases=(),
            sim_require_finite=True,
            sim_require_nnan=True,
            nc=nc,
        )
        return tuple(outs)

    devices = [d for d in jax.devices() if d.platform != "cpu"][:NCORES]
    if len(devices) < NCORES:
        devices = jax.devices()[:NCORES]
    assert len(devices) == NCORES, f"need {NCORES} neuron cores"
    mesh = Mesh(np.asarray(devices), ("core",))

    def spec(name):
        return (PartitionSpec("core") if name in SHARDED_INPUTS
                else PartitionSpec())

    in_specs = tuple(spec(n) for n in in_names) + \
        (PartitionSpec("core"),) * n_outs
    out_specs = (PartitionSpec("core"),) * n_outs
    sharded = jax.jit(
        shard_map(_body, mesh=mesh, in_specs=in_specs, out_specs=out_specs,
                  check_rep=False),
        donate_argnums=donate, keep_unused=True)

    def run(per_core_maps):
        args = []
        for name in in_names:
            if name in SHARDED_INPUTS:
                args.append(np.concatenate(
                    [m[name] for m in per_core_maps], 0))
            else:
                args.append(np.asarray(per_core_maps[0][name]))
        zeros = [np.zeros((NCORES * a.shape[0], *a.shape[1:]), a.dtype)
                 for a in (np.zeros(s.shape, s.dtype) for s in out_avals)]
        outs = sharded(*args, *zeros)
        res = []
        for c in range(NCORES):
            res.append({name: np.asarray(outs[i]).reshape(
                NCORES, *out_avals[i].shape)[c]
                for i, name in enumerate(out_names)})
        return res

    _CACHE["run"] = run
    _CACHE["sharded"] = sharded
    _CACHE["in_names"] = in_names
    _CACHE["out_avals"] = out_avals
    return run


def kernel(**inputs):
    use_f32r = os.environ.get("AK_F32", "") != "1"
    per_core = host_prep(inputs)
    run = _get_executor(use_f32r=use_f32r)
    results = run(per_core)
    outs = [np.transpose(r["out"], (0, 2, 1)) for r in results]
    return np.concatenate(outs, 0).astype(np.float32)


if __name__ == "__main__":
    pass



# revision 2
# speedup vs baseline: 30.6634x; 30.6634x over previous
"""Autoformer forward on 8 Trainium2 NeuronCores, data-parallel over batch."""
import math
import os
import sys

sys.path.insert(0, "/opt/trn_rl_repo")
import numpy as np
from contextlib import ExitStack

import concourse.bass as bass
import concourse.bacc as bacc
import concourse.mybir as mybir
from concourse.tile import TileContext
from concourse.bass_utils import run_bass_kernel_spmd

AL = mybir.AluOpType
AF = mybir.ActivationFunctionType
F32 = mybir.dt.float32
F32R = mybir.dt.float32r
U32 = mybir.dt.uint32
AX = mybir.AxisListType
DVE = mybir.EngineType.DVE

B, SEQ, LAB, PRED = 32, 720, 336, 720
CIN, D, H, DFF, EL, MA = 21, 512, 8, 2048, 2, 25
L1, L2 = SEQ, LAB + PRED            # 720, 1056
LF1, LF2 = L1 // 2 + 1, L2 // 2 + 1  # 361, 529
TOPK = 6
PAD = (MA - 1) // 2                 # 12
NCORES = 8
BC = B // NCORES                    # 4 batches per core
DC = D // 128                       # 4 feature chunks
DFC = DFF // 128                    # 16

TC1 = (L1 + 127) // 128             # 6   seq chunks (enc)
TC2 = (L2 + 127) // 128             # 9   seq chunks (dec)
NFC1 = (LF1 + 127) // 128           # 3   freq chunks (enc)
NFC2 = (LF2 + 127) // 128           # 5   freq chunks (dec)
NT1 = [(0, 360), (360, 360)]
NT2 = [(0, 352), (352, 352), (704, 352)]


def _ft_chunks(lf, nfc):
    return [(i * 128, min(128, lf - i * 128)) for i in range(nfc)]


FT1 = _ft_chunks(LF1, NFC1)
FT2 = _ft_chunks(LF2, NFC2)


# ---------------------------------------------------------------- host consts
def _dft_mats(L, LF):
    t = np.arange(L, dtype=np.float64)[:, None]
    f = np.arange(LF, dtype=np.float64)[None, :]
    ang = 2.0 * np.pi * f * t / L
    rows = 128 * ((L + 127) // 128)
    dc = np.zeros((rows, LF), np.float32)
    ds = np.zeros((rows, LF), np.float32)
    dc[:L] = np.cos(ang)
    ds[:L] = -np.sin(ang)
    return dc, ds


def _idft_mat(L, LF, nfc):
    # rows: chunks 0..nfc-1 = Sre (f), chunks nfc..2nfc-1 = Sim (f); corr scale 1/(L*D)
    t = np.arange(L, dtype=np.float64)[None, :]
    f = np.arange(LF, dtype=np.float64)[:, None]
    ang = 2.0 * np.pi * f * t / L
    w = np.full((LF, 1), 2.0)
    w[0, 0] = 1.0
    w[-1, 0] = 1.0
    scale = 1.0 / (L * D)
    icr = (w * np.cos(ang) * scale).astype(np.float32)
    ism = (-w * np.sin(ang) * scale).astype(np.float32)
    out = np.zeros((2 * nfc * 128, L), np.float32)
    out[:LF] = icr
    out[nfc * 128:nfc * 128 + LF] = ism
    return out


def _chunked_bias(b):
    # [C*128] -> [128, C] per-partition layout
    c = b.shape[0] // 128
    return np.ascontiguousarray(b.reshape(c, 128).T).astype(np.float32)


def host_prep(inp):
    """Return (shared weight/const map, per-core input maps)."""
    g = {}

    def wT(w):  # torch Linear weight [out,in] -> [in,out]
        return np.ascontiguousarray(np.asarray(w).T).astype(np.float32)

    # embeddings: combined [67, 512] (rows j*21+c from tok_W[o,c,j], + time rows)
    for pre, tok, tim in (("e", inp["enc_tok_W"], inp["enc_time_W"]),
                          ("d", inp["dec_tok_W"], inp["dec_time_W"])):
        tok = np.asarray(tok)  # [512, 21, 3]
        m = np.transpose(tok, (2, 1, 0)).reshape(63, D)  # row j*21+c
        t = np.asarray(tim)  # [512, 4]
        g[f"embW_{pre}"] = np.concatenate([m, t.T], 0).astype(np.float32)  # [67,512]

    for l in range(EL):
        for nm in ("q", "k", "v", "o"):
            g[f"eW{nm}{l}"] = wT(inp[f"enc_W{nm}"][l])
            g[f"eb{nm}{l}"] = np.asarray(inp[f"enc_b{nm}"][l]).astype(np.float32)[None, :]
        g[f"eW1{l}"] = wT(inp["enc_W1"][l])
        g[f"eb1{l}"] = _chunked_bias(np.asarray(inp["enc_b1"][l]))
        g[f"eW2{l}"] = wT(inp["enc_W2"][l])
        g[f"eb2{l}"] = _chunked_bias(np.asarray(inp["enc_b2"][l]))
        g[f"ebvC{l}"] = _chunked_bias(np.asarray(inp["enc_bv"][l]))
        g[f"eboC{l}"] = _chunked_bias(np.asarray(inp["enc_bo"][l]))
    for pre in ("ds", "dc"):
        for nm in ("q", "k", "v", "o"):
            g[f"{pre}W{nm}"] = wT(inp[f"{pre}_W{nm}"])
            g[f"{pre}b{nm}"] = np.asarray(inp[f"{pre}_b{nm}"]).astype(np.float32)[None, :]
        g[f"{pre}bvC"] = _chunked_bias(np.asarray(inp[f"{pre}_bv"]))
        g[f"{pre}boC"] = _chunked_bias(np.asarray(inp[f"{pre}_bo"]))
    g["dW1"] = wT(inp["dec_W1"])
    g["db1"] = _chunked_bias(np.asarray(inp["dec_b1"]))
    g["dW2"] = wT(inp["dec_W2"])
    g["db2"] = _chunked_bias(np.asarray(inp["dec_b2"]))

    tw = np.asarray(inp["dec_trend_W"])  # [21, 512, 3]
    g["trendW"] = np.transpose(tw, (2, 1, 0)).reshape(3 * D, CIN).astype(np.float32)
    g["projW"] = wT(inp["dec_proj_W"])  # [512, 21]
    g["projB"] = np.asarray(inp["dec_proj_b"]).astype(np.float32)[:, None]  # [21,1]
    g["enW"] = np.ascontiguousarray(
        np.asarray(inp["enc_norm_w"]).reshape(DC, 128).T).astype(np.float32)
    g["enB"] = np.ascontiguousarray(
        np.asarray(inp["enc_norm_b"]).reshape(DC, 128).T).astype(np.float32)
    g["dnW"] = np.ascontiguousarray(
        np.asarray(inp["dec_norm_w"]).reshape(DC, 128).T).astype(np.float32)
    g["dnB"] = np.ascontiguousarray(
        np.asarray(inp["dec_norm_b"]).reshape(DC, 128).T).astype(np.float32)

    g["ones512"] = np.ones((1, 512), np.float32)
    g["onescol"] = np.ones((128, 1), np.float32)
    g["zeros"] = np.zeros((128, 1056), np.float32)
    g["dftc1"], g["dfts1"] = _dft_mats(L1, LF1)
    g["idft1"] = _idft_mat(L1, LF1, NFC1)
    g["dftc2"], g["dfts2"] = _dft_mats(L2, LF2)
    g["idft2"] = _idft_mat(L2, LF2, NFC2)

    xT = np.transpose(np.asarray(inp["x_enc"]), (0, 2, 1)).astype(np.float32)
    mke = np.transpose(np.asarray(inp["x_mark_enc"]), (0, 2, 1)).astype(np.float32)
    mkd = np.transpose(np.asarray(inp["x_mark_dec"]), (0, 2, 1)).astype(np.float32)
    xT = np.ascontiguousarray(xT)
    mke = np.ascontiguousarray(mke)
    mkd = np.ascontiguousarray(mkd)

    per_core = []
    for c in range(NCORES):
        sl = slice(c * BC, (c + 1) * BC)
        m = dict(g)
        m["xT"] = np.ascontiguousarray(xT[sl])
        m["mkeT"] = np.ascontiguousarray(mke[sl])
        m["mkdT"] = np.ascontiguousarray(mkd[sl])
        per_core.append(m)
    return per_core


# ---------------------------------------------------------------- device build
def build_nc(use_f32r=True, sim=False):
    MMDT = F32R if use_f32r else F32
    GELU = AF.Identity if sim else AF.Gelu
    nc = bacc.Bacc(None, target_bir_lowering=False)

    dram = {}

    def din(name, shape, dt=MMDT):
        dram[name] = nc.dram_tensor(name, list(shape), dt, kind="ExternalInput")
        return dram[name]

    # inputs
    din("xT", (BC, CIN, L1))
    din("mkeT", (BC, 4, L1))
    din("mkdT", (BC, 4, L2))
    din("embW_e", (67, D))
    din("embW_d", (67, D))
    for l in range(EL):
        for nm in ("q", "k", "v", "o"):
            din(f"eW{nm}{l}", (D, D))
            din(f"eb{nm}{l}", (1, D))
        din(f"eW1{l}", (D, DFF))
        din(f"eb1{l}", (128, DFC), F32)
        din(f"eW2{l}", (DFF, D))
        din(f"eb2{l}", (128, DC), F32)
        din(f"ebvC{l}", (128, DC), F32)
        din(f"eboC{l}", (128, DC), F32)
    for pre in ("ds", "dc"):
        for nm in ("q", "k", "v", "o"):
            din(f"{pre}W{nm}", (D, D))
            din(f"{pre}b{nm}", (1, D))
        din(f"{pre}bvC", (128, DC), F32)
        din(f"{pre}boC", (128, DC), F32)
    din("dW1", (D, DFF))
    din("db1", (128, DFC), F32)
    din("dW2", (DFF, D))
    din("db2", (128, DC), F32)
    din("trendW", (3 * D, CIN))
    din("projW", (D, CIN))
    din("projB", (CIN, 1), F32)
    for nm in ("enW", "enB", "dnW", "dnB"):
        din(nm, (128, DC), F32)
    din("ones512", (1, 512))
    din("onescol", (128, 1))
    din("zeros", (128, 1056))
    din("dftc1", (TC1 * 128, LF1))
    din("dfts1", (TC1 * 128, LF1))
    din("idft1", (2 * NFC1 * 128, L1))
    din("dftc2", (TC2 * 128, LF2))
    din("dfts2", (TC2 * 128, LF2))
    din("idft2", (2 * NFC2 * 128, L2))

    out_d = nc.dram_tensor("out", [BC, CIN, PRED], F32, kind="ExternalOutput")

    # DRAM scratch
    enc_dram = nc.dram_tensor("enc_scratch", [BC, DC, 128, L1], MMDT)
    dec_dram = nc.dram_tensor("dec_scratch", [BC, DC, 128, L2], MMDT)
    tsum_dram = nc.dram_tensor("tsum_scratch", [BC, DC, 128, L2], F32)
    seas_dram = nc.dram_tensor("seas_scratch", [BC, CIN, LAB], MMDT)
    trendi_dram = nc.dram_tensor("trendi_scratch", [BC, CIN, L2], F32)

    with TileContext(nc) as tc, ExitStack() as top:
        cpool = top.enter_context(tc.tile_pool(name="consts", bufs=1))
        ones_row = cpool.tile([1, 512], MMDT)
        nc.sync.dma_start(ones_row, dram["ones512"][:])
        ones_col = cpool.tile([128, 1], MMDT)
        nc.sync.dma_start(ones_col, dram["onescol"][:])
        zeros_t = cpool.tile([128, 1056], MMDT)
        nc.sync.dma_start(zeros_t, dram["zeros"][:])

        # ---------------------------------------------------- helper closures
        def linear_T(ps_pool, out, X, W, bias, L, tcn):
            """out[128, tcn, 512] (seq-part) = X.T @ W + bias ; X[128,DC,L]."""
            if L % 128:
                nc.vector.tensor_copy(out[:, tcn - 1, :], zeros_t[:, 0:512])
            for mt in range(tcn):
                m = min(128, L - mt * 128)
                ps = ps_pool.tile([128, 512], F32, tag="mm")
                for kc in range(DC):
                    nc.tensor.matmul(ps[0:m, :], X[:, kc, mt * 128:mt * 128 + m],
                                     W[:, kc, :], start=(kc == 0), stop=False)
                nc.tensor.matmul(ps[0:m, :], ones_row[0:1, 0:m], bias,
                                 start=False, stop=True)
                nc.scalar.copy(out[0:m, mt, :], ps[0:m, :])

        def dft_S(ps_pool, tmp_pool, Sstk, q, k, dftc, dfts, fts, nfc, tck, tcq):
            """Sstk[128, 2*nfc, 1] f32r: stacked sum_c Qf*conj(Kf)."""
            nc.vector.tensor_copy(Sstk[:, :, 0], zeros_t[:, 0:Sstk.shape[1]])
            for ft, (f0, fm) in enumerate(fts):
                scr = {}
                for nm, mat, src, tcs in (("qr", dftc, q, tcq), ("qi", dfts, q, tcq),
                                          ("kr", dftc, k, tck), ("ki", dfts, k, tck)):
                    ps = ps_pool.tile([128, 512], F32, tag="pdft", bufs=2,
                                      name=f"pdft_{nm}")
                    for t in range(tcs):
                        nc.tensor.matmul(ps[0:fm, :], mat[:, t, f0:f0 + fm],
                                         src[:, t, :], start=(t == 0),
                                         stop=(t == tcs - 1))
                    sc = tmp_pool.tile([128, 512], F32, tag=f"s{nm}",
                                       name=f"s{nm}")
                    nc.scalar.copy(sc[0:fm, :], ps[0:fm, :])
                    scr[nm] = sc
                prod = tmp_pool.tile([128, 512], F32, tag="prod", bufs=2)
                cols = tmp_pool.tile([128, 4], F32, tag="cols", bufs=2)
                for ci, (xa, xb) in enumerate((("qr", "kr"), ("qi", "ki"),
                                               ("qi", "kr"), ("qr", "ki"))):
                    nc.vector.scalar_tensor_tensor(
                        prod[0:fm, :], scr[xa][0:fm, :], 1.0, scr[xb][0:fm, :],
                        op0=AL.bypass, op1=AL.mult,
                        accum_out=cols[0:fm, ci:ci + 1])
                nc.vector.tensor_tensor(Sstk[0:fm, ft, 0:1], cols[0:fm, 0:1],
                                        cols[0:fm, 1:2], AL.add)
                nc.vector.tensor_tensor(Sstk[0:fm, nfc + ft, 0:1], cols[0:fm, 2:3],
                                        cols[0:fm, 3:4], AL.subtract)

        def topk_tw(ps_pool, tmp_pool, Sstk, idft, nfc, L, nts):
            """corr -> (twb[128,8] f32, i8[1,8] u32)."""
            corr = tmp_pool.tile([1, L], F32, tag="corr")
            for nt, (n0, n) in enumerate(nts):
                psc = ps_pool.tile([1, 512], F32, tag="corrps")
                for j in range(2 * nfc):
                    nc.tensor.matmul(psc[:, 0:n], Sstk[:, j, 0:1],
                                     idft[:, j, n0:n0 + n],
                                     start=(j == 0), stop=(j == 2 * nfc - 1))
                nc.scalar.copy(corr[:, n0:n0 + n], psc[:, 0:n])
            w8 = tmp_pool.tile([1, 8], F32, tag="w8")
            i8 = tmp_pool.tile([1, 8], U32, tag="i8")
            nc.vector.max_with_indices(w8, i8, corr)
            e6 = tmp_pool.tile([1, 8], F32, tag="e6")
            nc.vector.memset(e6[:, TOPK:8], 0.0)
            nc.vector.tensor_scalar_sub(e6[:, 0:TOPK], w8[:, 0:TOPK], w8[:, 0:1])
            nc.scalar.activation(e6[:, 0:TOPK], e6[:, 0:TOPK], AF.Exp)
            ssum = tmp_pool.tile([1, 1], F32, tag="ssum")
            nc.vector.reduce_sum(ssum, e6[:, 0:TOPK], axis=AX.X)
            nc.vector.reciprocal(ssum, ssum)
            nc.vector.tensor_scalar_mul(e6[:, 0:TOPK], e6[:, 0:TOPK], ssum)
            twb = tmp_pool.tile([128, 8], F32, tag="twb")
            nc.gpsimd.partition_broadcast(twb, e6[0:1, :])
            return twb, i8

        def agg_delays(agg, vv, twb, i8, L):
            for kk in range(TOPK):
                dly = nc.values_load(i8[0:1, kk:kk + 1], min_val=0, max_val=L - 1,
                                     engines=[DVE], skip_runtime_bounds_check=True)
                for mc in range(DC):
                    src = vv[:, mc, bass.ds(dly, L)]
                    if kk == 0:
                        nc.vector.scalar_tensor_tensor(
                            agg[:, mc, :], src, twb[:, 0:1], src,
                            op0=AL.mult, op1=AL.bypass)
                    else:
                        nc.vector.scalar_tensor_tensor(
                            agg[:, mc, :], src, twb[:, kk:kk + 1], agg[:, mc, :],
                            op0=AL.mult, op1=AL.add)

        def out_proj_residual(ps_pool, X, agg, W, boC, nts):
            """X += agg.T@W + bo  (F-layout, in place)."""
            for mc in range(DC):
                for (n0, n) in nts:
                    ps = ps_pool.tile([128, 512], F32, tag="mm")
                    for kc in range(DC):
                        nc.tensor.matmul(ps[:, 0:n], W[:, kc, mc * 128:(mc + 1) * 128],
                                         agg[:, kc, n0:n0 + n],
                                         start=(kc == 0), stop=(kc == DC - 1))
                    nc.vector.scalar_tensor_tensor(
                        X[:, mc, n0:n0 + n], ps[:, 0:n], boC[:, mc:mc + 1],
                        X[:, mc, n0:n0 + n], op0=AL.add, op1=AL.add)

        def decomp(tmp_pool, X, L, chunks=DC, trend_to=None, trend_accum=None,
                   trend_dram_b=None, db=1):
            """X <- X - mavg(X) in place; optionally emit trend (mavg)."""
            for mc in range(chunks):
                xp = tmp_pool.tile([128, L + 2 * PAD], F32, tag="xp", bufs=db)
                nc.scalar.copy(xp[:, PAD:PAD + L], X[:, mc, :])
                nc.vector.tensor_copy(xp[:, 0:PAD],
                                      X[:, mc, 0:1].to_broadcast([128, PAD]))
                nc.vector.tensor_copy(xp[:, PAD + L:],
                                      X[:, mc, L - 1:L].to_broadcast([128, PAD]))
                cs = tmp_pool.tile([128, L + 2 * PAD + 1], F32, tag="cs", bufs=db)
                nc.vector.memset(cs[:, 0:1], 0.0)
                nc.vector.tensor_tensor_scan(cs[:, 1:], xp, xp, 0.0, AL.add, AL.bypass)
                dt = tmp_pool.tile([128, L], F32, tag="dt", bufs=2)
                nc.vector.tensor_tensor(dt, cs[:, MA:MA + L], cs[:, 0:L], AL.subtract)
                if trend_to is not None:
                    nc.vector.tensor_scalar_mul(trend_to[:, mc, :], dt, 1.0 / MA)
                if trend_accum is not None:
                    tt = tmp_pool.tile([128, L], F32, tag="taccum", bufs=2)
                    nc.vector.tensor_scalar_mul(tt, dt, 1.0 / MA)
                    nc.gpsimd.dma_start(tsum_dram[trend_dram_b, mc], tt,
                                        accum_op=(AL.add if trend_accum == "add"
                                                  else AL.bypass))
                nc.vector.scalar_tensor_tensor(X[:, mc, :], dt, -1.0 / MA,
                                               X[:, mc, :], op0=AL.mult, op1=AL.add)

        def ffn(ps_pool, tmp_pool, X, W1, b1C, W2, b2C, L, nts):
            """X += gelu(X@W1+b1)@W2+b2 in place (F-layout)."""
            h = tmp_pool.tile([128, DFC, L], MMDT, tag="h")
            for mh in range(DFC):
                for (n0, n) in nts:
                    ps = ps_pool.tile([128, 512], F32, tag="mm")
                    for kc in range(DC):
                        nc.tensor.matmul(ps[:, 0:n], W1[:, kc, mh * 128:(mh + 1) * 128],
                                         X[:, kc, n0:n0 + n],
                                         start=(kc == 0), stop=(kc == DC - 1))
                    nc.scalar.activation(h[:, mh, n0:n0 + n], ps[:, 0:n], GELU,
                                         bias=b1C[:, mh:mh + 1])
            for mc in range(DC):
                for (n0, n) in nts:
                    ps = ps_pool.tile([128, 512], F32, tag="mm")
                    for kh in range(DFC):
                        nc.tensor.matmul(ps[:, 0:n], W2[:, kh, mc * 128:(mc + 1) * 128],
                                         h[:, kh, n0:n0 + n],
                                         start=(kh == 0), stop=(kh == DFC - 1))
                    nc.vector.scalar_tensor_tensor(
                        X[:, mc, n0:n0 + n], ps[:, 0:n], b2C[:, mc:mc + 1],
                        X[:, mc, n0:n0 + n], op0=AL.add, op1=AL.add)

        def layernorm(ps_pool, tmp_pool, X, Xln, L, nts, wD, bD):
            """Xln = LN(X) over feature dim (partition dim, DC chunks)."""
            xsq = tmp_pool.tile([128, DC, L], MMDT, tag="xsq")
            for mc in range(DC):
                nc.scalar.activation(xsq[:, mc, :], X[:, mc, :], AF.Square)
            pmu = ps_pool.tile([1, len(nts), 512], F32, tag="pmu")
            psq = ps_pool.tile([1, len(nts), 512], F32, tag="psq")
            for nt, (n0, n) in enumerate(nts):
                for kc in range(DC):
                    st, sp = (kc == 0), (kc == DC - 1)
                    nc.tensor.matmul(pmu[:, nt, 0:n], ones_col, X[:, kc, n0:n0 + n],
                                     start=st, stop=sp)
                    nc.tensor.matmul(psq[:, nt, 0:n], ones_col, xsq[:, kc, n0:n0 + n],
                                     start=st, stop=sp)
            stats = tmp_pool.tile([1, 2 * L], F32, tag="stats")
            mu, rstd = stats[:, 0:L], stats[:, L:2 * L]
            for nt, (n0, n) in enumerate(nts):
                nc.vector.tensor_scalar_mul(mu[:, n0:n0 + n], pmu[:, nt, 0:n], 1.0 / D)
                nc.vector.tensor_scalar_mul(rstd[:, n0:n0 + n], psq[:, nt, 0:n], 1.0 / D)
            musq = tmp_pool.tile([1, L], F32, tag="musq")
            nc.vector.tensor_tensor(musq, mu, mu, AL.mult)
            nc.vector.tensor_tensor(rstd, rstd, musq, AL.subtract)
            nc.vector.tensor_scalar_add(rstd, rstd, 1e-5)
            nc.scalar.activation(rstd, rstd, AF.Sqrt)
            nc.vector.reciprocal(rstd, rstd)
            stb = tmp_pool.tile([128, 2 * L], F32, tag="stb")
            nc.gpsimd.partition_broadcast(stb, stats[0:1, :])
            t = tmp_pool.tile([128, L], F32, tag="lnt")
            for mc in range(DC):
                nc.vector.tensor_tensor(t, X[:, mc, :], stb[:, 0:L], AL.subtract)
                nc.vector.tensor_tensor(t, t, stb[:, L:2 * L], AL.mult)
                nc.vector.tensor_scalar_mul(t, t, wD[:, mc:mc + 1])
                nc.vector.tensor_scalar_add(Xln[:, mc, :], t, bD[:, mc:mc + 1])

        # ======================================================== ENCODER
        with tc.tile_pool(name="acts", bufs=1) as apool:
            enc_acts = [apool.tile([128, DC, L1], MMDT, tag=f"enc{b}",
                                   name=f"enc_acts{b}")
                        for b in range(BC)]

            # ---- P0: embedding + init decomposition
            with nc.named_scope("P0_embed"), \
                 tc.tile_pool(name="p0t", bufs=2) as tp, \
                 tc.tile_pool(name="p0w", bufs=1) as wp, \
                 tc.tile_pool(name="p0ps", bufs=4, space="PSUM") as psp:
                embW = wp.tile([67, D], MMDT)
                nc.sync.dma_start(embW, dram["embW_e"][:])
                for b in range(BC):
                    win = tp.tile([67, L1], MMDT, tag="win")
                    nc.sync.dma_start(win[0:CIN, 1:L1], dram["xT"][b, :, 0:L1 - 1])
                    nc.sync.dma_start(win[0:CIN, 0:1], dram["xT"][b, :, L1 - 1:L1])
                    nc.sync.dma_start(win[CIN:2 * CIN, :], dram["xT"][b])
                    nc.sync.dma_start(win[2 * CIN:3 * CIN, 0:L1 - 1],
                                      dram["xT"][b, :, 1:L1])
                    nc.sync.dma_start(win[2 * CIN:3 * CIN, L1 - 1:L1],
                                      dram["xT"][b, :, 0:1])
                    nc.sync.dma_start(win[63:67, :], dram["mkeT"][b])
                    for mc in range(DC):
                        for (n0, n) in NT1:
                            ps = psp.tile([128, 512], F32, tag="mm")
                            nc.tensor.matmul(ps[:, 0:n],
                                             embW[:, mc * 128:(mc + 1) * 128],
                                             win[:, n0:n0 + n], start=True, stop=True)
                            nc.scalar.copy(enc_acts[b][:, mc, n0:n0 + n], ps[:, 0:n])

                # init series_decomp of x_enc (packed [84, .])
                xe = tp.tile([84, L1], MMDT, tag="xe")
                for b in range(BC):
                    nc.sync.dma_start(xe[b * CIN:(b + 1) * CIN, :], dram["xT"][b])
                xp = tp.tile([84, L1 + 2 * PAD], F32, tag="ixp")
                nc.scalar.copy(xp[:, PAD:PAD + L1], xe)
                nc.vector.tensor_copy(xp[:, 0:PAD], xe[:, 0:1].to_broadcast([84, PAD]))
                nc.vector.tensor_copy(xp[:, PAD + L1:],
                                      xe[:, L1 - 1:L1].to_broadcast([84, PAD]))
                cs = tp.tile([84, L1 + 2 * PAD + 1], F32, tag="ics")
                nc.vector.memset(cs[:, 0:1], 0.0)
                nc.vector.tensor_tensor_scan(cs[:, 1:], xp, xp, 0.0, AL.add, AL.bypass)
                dt = tp.tile([84, L1], F32, tag="idt")
                nc.vector.tensor_tensor(dt, cs[:, MA:MA + L1], cs[:, 0:L1], AL.subtract)
                seas = tp.tile([84, L1], MMDT, tag="iseas")
                nc.vector.scalar_tensor_tensor(seas, dt, -1.0 / MA, xe,
                                               op0=AL.mult, op1=AL.add)
                trend = tp.tile([84, L1], F32, tag="itrend")
                nc.vector.tensor_scalar_mul(trend, dt, 1.0 / MA)
                mean = tp.tile([84, 1], F32, tag="imean")
                nc.vector.reduce_sum(mean, xe, axis=AX.X)
                nc.vector.tensor_scalar_mul(mean, mean, 1.0 / L1)
                meanb = tp.tile([84, PRED], F32, tag="imeanb")
                nc.vector.tensor_copy(meanb, mean.to_broadcast([84, PRED]))
                for b in range(BC):
                    sl = slice(b * CIN, (b + 1) * CIN)
                    nc.sync.dma_start(seas_dram[b], seas[sl, L1 - LAB:L1])
                    nc.sync.dma_start(trendi_dram[b, :, 0:LAB], trend[sl, L1 - LAB:L1])
                    nc.sync.dma_start(trendi_dram[b, :, LAB:L2], meanb[sl, :])

            # ---- P1/P2: encoder layers
            if True:
                for l in range(EL):
                    with nc.named_scope(f"enc{l}_att"), \
                         tc.tile_pool(name="dft1", bufs=1) as dft1p, \
                         tc.tile_pool(name="eatw", bufs=1) as wp, \
                         tc.tile_pool(name="eatt", bufs=1) as tp, \
                         tc.tile_pool(name="eatps", bufs=1, space="PSUM") as psp, \
                         tc.tile_pool(name="eatps2", bufs=3, space="PSUM") as psp2:
                        dftc1 = dft1p.tile([128, TC1, LF1], MMDT, tag="dftc1")
                        dfts1 = dft1p.tile([128, TC1, LF1], MMDT, tag="dfts1")
                        idft1 = dft1p.tile([128, 2 * NFC1, L1], MMDT, tag="idft1")
                        nc.sync.dma_start(dftc1, dram["dftc1"][:].rearrange(
                            "(c p) f -> p c f", p=128))
                        nc.sync.dma_start(dfts1, dram["dfts1"][:].rearrange(
                            "(c p) f -> p c f", p=128))
                        nc.sync.dma_start(idft1, dram["idft1"][:].rearrange(
                            "(c p) f -> p c f", p=128))
                        Ws = {}
                        for nm in ("q", "k", "v", "o"):
                            Ws[nm] = wp.tile([128, DC, D], MMDT, tag=f"W{nm}",
                                             name=f"W{nm}")
                            nc.sync.dma_start(Ws[nm], dram[f"eW{nm}{l}"][:].rearrange(
                                "(c p) f -> p c f", p=128))
                        bq = wp.tile([1, D], MMDT, tag="bq")
                        bk = wp.tile([1, D], MMDT, tag="bk")
                        nc.sync.dma_start(bq, dram[f"ebq{l}"][:])
                        nc.sync.dma_start(bk, dram[f"ebk{l}"][:])
                        bvC = wp.tile([128, DC], F32, tag="bvC")
                        boC = wp.tile([128, DC], F32, tag="boC")
                        nc.sync.dma_start(bvC, dram[f"ebvC{l}"][:])
                        nc.sync.dma_start(boC, dram[f"eboC{l}"][:])
                        for b in range(BC):
                            X = enc_acts[b]
                            q = tp.tile([128, TC1, 512], MMDT, tag="q")
                            k = tp.tile([128, TC1, 512], MMDT, tag="k", bufs=2)
                            linear_T(psp2, q, X, Ws["q"], bq, L1, TC1)
                            linear_T(psp2, k, X, Ws["k"], bk, L1, TC1)
                            Sstk = tp.tile([128, 2 * NFC1, 1], MMDT, tag="Sstk")
                            dft_S(psp, tp, Sstk, q, k, dftc1, dfts1, FT1, NFC1,
                                  TC1, TC1)
                            twb, i8 = topk_tw(psp, tp, Sstk, idft1, NFC1, L1, NT1)
                            vv = tp.tile([128, DC, 2 * L1], F32, tag="q")
                            for mc in range(DC):
                                for (n0, n) in NT1:
                                    ps = psp2.tile([128, 512], F32, tag="mm")
                                    for kc in range(DC):
                                        nc.tensor.matmul(
                                            ps[:, 0:n],
                                            Ws["v"][:, kc, mc * 128:(mc + 1) * 128],
                                            X[:, kc, n0:n0 + n],
                                            start=(kc == 0), stop=(kc == DC - 1))
                                    nc.scalar.activation(vv[:, mc, n0:n0 + n],
                                                         ps[:, 0:n], AF.Identity,
                                                         bias=bvC[:, mc:mc + 1])
                                    nc.scalar.activation(vv[:, mc, L1 + n0:L1 + n0 + n],
                                                         ps[:, 0:n], AF.Identity,
                                                         bias=bvC[:, mc:mc + 1])
                            agg = tp.tile([128, DC, L1], MMDT, tag="k", bufs=2)
                            agg_delays(agg, vv, twb, i8, L1)
                            out_proj_residual(psp2, X, agg, Ws["o"], boC, NT1)
                            decomp(tp, X, L1)

                    with nc.named_scope(f"enc{l}_ffn"), \
                         tc.tile_pool(name="effw", bufs=1) as wp, \
                         tc.tile_pool(name="efft", bufs=1) as tp, \
                         tc.tile_pool(name="effps", bufs=6, space="PSUM") as psp:
                        W1 = wp.tile([128, DC, DFF], MMDT, tag="W1")
                        W2 = wp.tile([128, DFC, D], MMDT, tag="W2")
                        nc.sync.dma_start(W1, dram[f"eW1{l}"][:].rearrange(
                            "(c p) f -> p c f", p=128))
                        nc.sync.dma_start(W2, dram[f"eW2{l}"][:].rearrange(
                            "(c p) f -> p c f", p=128))
                        b1C = wp.tile([128, DFC], F32, tag="b1C")
                        b2C = wp.tile([128, DC], F32, tag="b2C")
                        nc.sync.dma_start(b1C, dram[f"eb1{l}"][:])
                        nc.sync.dma_start(b2C, dram[f"eb2{l}"][:])
                        for b in range(BC):
                            ffn(psp, tp, enc_acts[b], W1, b1C, W2, b2C, L1, NT1)
                            decomp(tp, enc_acts[b], L1, db=2)

            # ---- P3: final encoder LN -> enc_dram
            with nc.named_scope("enc_ln"), \
                 tc.tile_pool(name="lnt", bufs=1) as tp, \
                 tc.tile_pool(name="lnw", bufs=1) as wp, \
                 tc.tile_pool(name="lnps", bufs=1, space="PSUM") as psp:
                enW = wp.tile([128, DC], F32, tag="enW")
                enB = wp.tile([128, DC], F32, tag="enB")
                nc.sync.dma_start(enW, dram["enW"][:])
                nc.sync.dma_start(enB, dram["enB"][:])
                for b in range(BC):
                    xln = tp.tile([128, DC, L1], MMDT, tag="xln")
                    layernorm(psp, tp, enc_acts[b], xln, L1, NT1, enW, enB)
                    for mc in range(DC):
                        nc.sync.dma_start(enc_dram[b, mc], xln[:, mc, :])

        # ======================================================== DECODER
        # ---- P4: decoder embedding -> dec_dram
        with nc.named_scope("dec_embed"), \
             tc.tile_pool(name="p4t", bufs=2) as tp, \
             tc.tile_pool(name="p4w", bufs=1) as wp, \
             tc.tile_pool(name="p4ps", bufs=4, space="PSUM") as psp:
            embW = wp.tile([67, D], MMDT)
            nc.sync.dma_start(embW, dram["embW_d"][:])
            for b in range(BC):
                win = tp.tile([67, L2], MMDT, tag="win2")
                nc.vector.tensor_copy(win[0:63, :], zeros_t[0:63, 0:L2])
                nc.sync.dma_start(win[0:CIN, 1:LAB + 1], seas_dram[b])
                nc.sync.dma_start(win[CIN:2 * CIN, 0:LAB], seas_dram[b])
                nc.sync.dma_start(win[2 * CIN:3 * CIN, 0:LAB - 1],
                                  seas_dram[b, :, 1:LAB])
                nc.sync.dma_start(win[2 * CIN:3 * CIN, L2 - 1:L2],
                                  seas_dram[b, :, 0:1])
                nc.sync.dma_start(win[63:67, :], dram["mkdT"][b])
                for mc in range(DC):
                    for (n0, n) in NT2:
                        ps = psp.tile([128, 512], F32, tag="mm")
                        nc.tensor.matmul(ps[:, 0:n], embW[:, mc * 128:(mc + 1) * 128],
                                         win[:, n0:n0 + n], start=True, stop=True)
                        xpart = tp.tile([128, 512], MMDT, tag="xpart")
                        nc.scalar.copy(xpart[:, 0:n], ps[:, 0:n])
                        nc.sync.dma_start(dec_dram[b, mc, :, n0:n0 + n], xpart[:, 0:n])

        # ---- P5/P6: decoder attentions
        for phase, pre in (("self", "ds"), ("cross", "dc")):
            with nc.named_scope(f"dec_{phase}"), \
                 tc.tile_pool(name="datw", bufs=1) as wp, \
                 tc.tile_pool(name="datt", bufs=1) as tp, \
                 tc.tile_pool(name="dft2", bufs=1) as dp, \
                 tc.tile_pool(name="datps", bufs=1, space="PSUM") as psp, \
                 tc.tile_pool(name="datps2", bufs=3, space="PSUM") as psp2:
                Ws = {}
                for nm in ("q", "k", "v", "o"):
                    Ws[nm] = wp.tile([128, DC, D], MMDT, tag=f"W{nm}",
                                     name=f"W{nm}")
                    nc.sync.dma_start(Ws[nm], dram[f"{pre}W{nm}"][:].rearrange(
                        "(c p) f -> p c f", p=128))
                bq = wp.tile([1, D], MMDT, tag="bq")
                bk = wp.tile([1, D], MMDT, tag="bk")
                nc.sync.dma_start(bq, dram[f"{pre}bq"][:])
                nc.sync.dma_start(bk, dram[f"{pre}bk"][:])
                bvC = wp.tile([128, DC], F32, tag="bvC")
                boC = wp.tile([128, DC], F32, tag="boC")
                nc.sync.dma_start(bvC, dram[f"{pre}bvC"][:])
                nc.sync.dma_start(boC, dram[f"{pre}boC"][:])
                kvsrc_chunks = TC2 if phase == "self" else TC1
                for b in range(BC):
                    X = tp.tile([128, DC, L2], MMDT, tag="Xd")
                    for mc in range(DC):
                        nc.sync.dma_start(X[:, mc, :], dec_dram[b, mc])
                    if phase == "self":
                        KV = X
                    else:
                        KV = tp.tile([128, DC, L1], MMDT, tag="KV")
                        for mc in range(DC):
                            nc.sync.dma_start(KV[:, mc, :], enc_dram[b, mc])
                    q = tp.tile([128, TC2, 512], MMDT, tag="q2")
                    k = tp.tile([128, TC2, 512], MMDT, tag="k2")
                    linear_T(psp2, q, X, Ws["q"], bq, L2, TC2)
                    linear_T(psp2, k, KV, Ws["k"], bk,
                             L2 if phase == "self" else L1, kvsrc_chunks)
                    dftcs = dp.tile([128, TC2, 2 * LF2], MMDT, tag="dftbig")
                    nc.sync.dma_start(dftcs[:, :, 0:LF2], dram["dftc2"][:].rearrange(
                        "(c p) f -> p c f", p=128))
                    nc.sync.dma_start(dftcs[:, :, LF2:], dram["dfts2"][:].rearrange(
                        "(c p) f -> p c f", p=128))
                    Sstk = tp.tile([128, 2 * NFC2, 1], MMDT, tag="Sstk2")
                    dft_S(psp, tp, Sstk, q, k, dftcs[:, :, 0:LF2],
                          dftcs[:, :, LF2:2 * LF2], FT2, NFC2, kvsrc_chunks, TC2)
                    idft2 = dp.tile([128, 2 * NFC2, L2], MMDT, tag="dftbig")
                    nc.sync.dma_start(idft2, dram["idft2"][:].rearrange(
                        "(c p) f -> p c f", p=128))
                    twb, i8 = topk_tw(psp, tp, Sstk, idft2, NFC2, L2, NT2)
                    VL = 2 * L2
                    vv = tp.tile([128, DC, VL], F32, tag="q2")
                    if phase == "cross":
                        nc.vector.memset(vv, 0.0)
                    kvL = L2 if phase == "self" else L1
                    kvNT = NT2 if phase == "self" else NT1
                    for mc in range(DC):
                        for (n0, n) in kvNT:
                            ps = psp2.tile([128, 512], F32, tag="mm")
                            for kc in range(DC):
                                nc.tensor.matmul(
                                    ps[:, 0:n],
                                    Ws["v"][:, kc, mc * 128:(mc + 1) * 128],
                                    KV[:, kc, n0:n0 + n],
                                    start=(kc == 0), stop=(kc == DC - 1))
                            nc.scalar.activation(vv[:, mc, n0:n0 + n], ps[:, 0:n],
                                                 AF.Identity, bias=bvC[:, mc:mc + 1])
                            nc.scalar.activation(vv[:, mc, L2 + n0:L2 + n0 + n],
                                                 ps[:, 0:n], AF.Identity,
                                                 bias=bvC[:, mc:mc + 1])
                    agg = tp.tile([128, DC, L2], MMDT, tag="k2")
                    agg_delays(agg, vv, twb, i8, L2)
                    out_proj_residual(psp2, X, agg, Ws["o"], boC, NT2)
                    decomp(tp, X, L2, trend_accum=("bypass" if phase == "self"
                                                   else "add"), trend_dram_b=b)
                    for mc in range(DC):
                        nc.sync.dma_start(dec_dram[b, mc], X[:, mc, :])

        # ---- P7: decoder FFN
        with nc.named_scope("dec_ffn"), \
             tc.tile_pool(name="dffw", bufs=1) as wp, \
             tc.tile_pool(name="dfft", bufs=1) as tp, \
             tc.tile_pool(name="dffps", bufs=6, space="PSUM") as psp:
            W1 = wp.tile([128, DC, DFF], MMDT, tag="W1")
            W2 = wp.tile([128, DFC, D], MMDT, tag="W2")
            nc.sync.dma_start(W1, dram["dW1"][:].rearrange("(c p) f -> p c f", p=128))
            nc.sync.dma_start(W2, dram["dW2"][:].rearrange("(c p) f -> p c f", p=128))
            b1C = wp.tile([128, DFC], F32, tag="b1C")
            b2C = wp.tile([128, DC], F32, tag="b2C")
            nc.sync.dma_start(b1C, dram["db1"][:])
            nc.sync.dma_start(b2C, dram["db2"][:])
            for b in range(BC):
                X = tp.tile([128, DC, L2], MMDT, tag="Xd")
                for mc in range(DC):
                    nc.sync.dma_start(X[:, mc, :], dec_dram[b, mc])
                ffn(psp, tp, X, W1, b1C, W2, b2C, L2, NT2)
                decomp(tp, X, L2, trend_accum="add", trend_dram_b=b, db=2)
                for mc in range(DC):
                    nc.sync.dma_start(dec_dram[b, mc], X[:, mc, :])

        # ---- P8: final (LN + proj, trend conv, combine)
        with nc.named_scope("final"), \
             tc.tile_pool(name="fint", bufs=1) as tp, \
             tc.tile_pool(name="finw", bufs=1) as wp, \
             tc.tile_pool(name="finps", bufs=1, space="PSUM") as psp, \
             tc.tile_pool(name="finps2", bufs=2, space="PSUM") as psp2:
            trendW = wp.tile([128, 3 * DC, CIN], MMDT, tag="trendW")
            nc.sync.dma_start(trendW, dram["trendW"][:].rearrange(
                "(c p) f -> p c f", p=128))
            projW = wp.tile([128, DC, CIN], MMDT, tag="projW")
            nc.sync.dma_start(projW, dram["projW"][:].rearrange(
                "(c p) f -> p c f", p=128))
            projB = wp.tile([CIN, 1], F32, tag="projB")
            nc.sync.dma_start(projB, dram["projB"][:])
            dnW = wp.tile([128, DC], F32, tag="dnW")
            dnB = wp.tile([128, DC], F32, tag="dnB")
            nc.sync.dma_start(dnW, dram["dnW"][:])
            nc.sync.dma_start(dnB, dram["dnB"][:])
            for b in range(BC):
                X = tp.tile([128, DC, L2], MMDT, tag="Xd")
                for mc in range(DC):
                    nc.sync.dma_start(X[:, mc, :], dec_dram[b, mc])
                xln = tp.tile([128, DC, L2], MMDT, tag="xln2")
                layernorm(psp, tp, X, xln, L2, NT2, dnW, dnB)
                seasonal = tp.tile([CIN, L2], F32, tag="seasonal")
                for (n0, n) in NT2:
                    ps = psp2.tile([CIN, 512], F32, tag="sm")
                    for kc in range(DC):
                        nc.tensor.matmul(ps[:, 0:n], projW[:, kc, :],
                                         xln[:, kc, n0:n0 + n],
                                         start=(kc == 0), stop=(kc == DC - 1))
                    nc.scalar.activation(seasonal[:, n0:n0 + n], ps[:, 0:n],
                                         AF.Identity, bias=projB)
                # trend conv windows [12 chunks, L2] circular
                tsum = tp.tile([128, DC, L2], F32, tag="tsum")
                for mc in range(DC):
                    nc.sync.dma_start(tsum[:, mc, :], tsum_dram[b, mc])
                winT = tp.tile([128, 3 * DC, L2], MMDT, tag="winT")
                for mc in range(DC):
                    # j=0: shift -1 ; j=1: center ; j=2: shift +1 (circular)
                    nc.scalar.copy(winT[:, mc, 1:L2], tsum[:, mc, 0:L2 - 1])
                    nc.scalar.copy(winT[:, mc, 0:1], tsum[:, mc, L2 - 1:L2])
                    nc.scalar.copy(winT[:, DC + mc, :], tsum[:, mc, :])
                    nc.scalar.copy(winT[:, 2 * DC + mc, 0:L2 - 1], tsum[:, mc, 1:L2])
                    nc.scalar.copy(winT[:, 2 * DC + mc, L2 - 1:L2], tsum[:, mc, 0:1])
                trendi = tp.tile([CIN, L2], F32, tag="trendi")
                nc.sync.dma_start(trendi, trendi_dram[b])
                outt = tp.tile([CIN, PRED], F32, tag="outt")
                for nt, (n0, n) in enumerate(NT2):
                    ps = psp2.tile([CIN, 512], F32, tag="sm")
                    for j in range(3 * DC):
                        nc.tensor.matmul(ps[:, 0:n], trendW[:, j, :],
                                         winT[:, j, n0:n0 + n],
                                         start=(j == 0), stop=(j == 3 * DC - 1))
                    trend_sl = tp.tile([CIN, 512], F32, tag="trend_sl")
                    nc.vector.tensor_tensor(trend_sl[:, 0:n], ps[:, 0:n],
                                            trendi[:, n0:n0 + n], AL.add)
                    # add seasonal, write PRED slice (cols >= LAB)
                    lo = max(n0, LAB)
                    hi = n0 + n
                    if hi > lo:
                        nc.vector.tensor_tensor(
                            outt[:, lo - LAB:hi - LAB], trend_sl[:, lo - n0:hi - n0],
                            seasonal[:, lo:hi], AL.add)
                nc.sync.dma_start(out_d[b], outt)

    nc.compile()
    return nc


# ---------------------------------------------------------------- entry point
_CACHE = {}
LAST_EXEC_NS = [None]
SHARDED_INPUTS = ("xT", "mkeT", "mkdT")


def _get_executor(use_f32r=True):
    if "run" in _CACHE:
        return _CACHE["run"]
    import jax
    from jax.sharding import Mesh, PartitionSpec
    try:
        from jax.experimental.shard_map import shard_map
    except Exception:
        from jax.shard_map import shard_map
    from concourse import bass2jax

    bass2jax.install_neuronx_cc_hook()
    nc = build_nc(use_f32r=use_f32r)
    _CACHE["nc"] = nc

    part_name = (nc.partition_id_tensor.name
                 if nc.partition_id_tensor else None)
    in_names, out_names, out_avals = [], [], []
    for alloc in nc.m.functions[0].allocations:
        if not isinstance(alloc, mybir.MemoryLocationSet):
            continue
        name = alloc.memorylocations[0].name
        if alloc.kind == "ExternalInput":
            if name != part_name:
                in_names.append(name)
        elif alloc.kind == "ExternalOutput":
            out_names.append(name)
            out_avals.append(jax.core.ShapedArray(
                tuple(alloc.tensor_shape), mybir.dt.np(alloc.dtype)))
    n_params = len(in_names)
    n_outs = len(out_names)
    all_names = list(in_names) + list(out_names)
    if part_name is not None:
        all_names.append(part_name)

    donate = tuple(range(n_params, n_params + n_outs))

    def _body(*args):
        operands = list(args)
        if part_name is not None:
            operands.append(bass2jax.partition_id_tensor())
        outs = bass2jax._bass_exec_p.bind(
            *operands,
            out_avals=tuple(out_avals),
            in_names=tuple(all_names),
            out_names=tuple(out_names),
            lowering_input_output_aliases=(),
            sim_require_finite=True,
            sim_require_nnan=True,
            nc=nc,
        )
        return tuple(outs)

    devices = [d for d in jax.devices() if d.platform != "cpu"][:NCORES]
    if len(devices) < NCORES:
        devices = jax.devices()[:NCORES]
    assert len(devices) == NCORES, f"need {NCORES} neuron cores"
    mesh = Mesh(np.asarray(devices), ("core",))

    def spec(name):
        return (PartitionSpec("core") if name in SHARDED_INPUTS
                else PartitionSpec())

    in_specs = tuple(spec(n) for n in in_names) + \
        (PartitionSpec("core"),) * n_outs
    out_specs = (PartitionSpec("core"),) * n_outs
    sharded = jax.jit(
        shard_map(_body, mesh=mesh, in_specs=in_specs, out_specs=out_specs,
                  check_rep=False),
        donate_argnums=donate, keep_unused=True)

    def run(per_core_maps):
        args = []
        for name in in_names:
            if name in SHARDED_INPUTS:
                args.append(np.concatenate(
                    [m[name] for m in per_core_maps], 0))
            else:
                args.append(np.asarray(per_core_maps[0][name]))
        zeros = [np.zeros((NCORES * a.shape[0], *a.shape[1:]), a.dtype)
                 for a in (np.zeros(s.shape, s.dtype) for s in out_avals)]
        outs = sharded(*args, *zeros)
        res = []
        for c in range(NCORES):
            res.append({name: np.asarray(outs[i]).reshape(
                NCORES, *out_avals[i].shape)[c]
                for i, name in enumerate(out_names)})
        return res

    _CACHE["run"] = run
    _CACHE["sharded"] = sharded
    _CACHE["in_names"] = in_names
    _CACHE["out_avals"] = out_avals
    return run


def kernel(**inputs):
    use_f32r = os.environ.get("AK_F32", "") != "1"
    per_core = host_prep(inputs)
    run = _get_executor(use_f32r=use_f32r)
    results = run(per_core)
    outs = [np.transpose(r["out"], (0, 2, 1)) for r in results]
    return np.concatenate(outs, 0).astype(np.float32)


if __name__ == "__main__":
    pass



# revision 41
# speedup vs baseline: 31.1322x; 1.0153x over previous
"""Autoformer forward on 8 Trainium2 NeuronCores, data-parallel over batch."""
import math
import os
import sys

sys.path.insert(0, "/opt/trn_rl_repo")
import numpy as np
from contextlib import ExitStack

import concourse.bass as bass
import concourse.bacc as bacc
import concourse.mybir as mybir
from concourse.tile import TileContext
from concourse.bass_utils import run_bass_kernel_spmd

AL = mybir.AluOpType
AF = mybir.ActivationFunctionType
F32 = mybir.dt.float32
F32R = mybir.dt.float32r
U32 = mybir.dt.uint32
AX = mybir.AxisListType
DVE = mybir.EngineType.DVE

B, SEQ, LAB, PRED = 32, 720, 336, 720
CIN, D, H, DFF, EL, MA = 21, 512, 8, 2048, 2, 25
L1, L2 = SEQ, LAB + PRED            # 720, 1056
LF1, LF2 = L1 // 2 + 1, L2 // 2 + 1  # 361, 529
TOPK = 6
PAD = (MA - 1) // 2                 # 12
NCORES = 8
BC = B // NCORES                    # 4 batches per core
DC = D // 128                       # 4 feature chunks
DFC = DFF // 128                    # 16

TC1 = (L1 + 127) // 128             # 6   seq chunks (enc)
TC2 = (L2 + 127) // 128             # 9   seq chunks (dec)
NFC1 = (LF1 + 127) // 128           # 3   freq chunks (enc)
NFC2 = (LF2 + 127) // 128           # 5   freq chunks (dec)
NT1 = [(0, 360), (360, 360)]
NT2 = [(0, 352), (352, 352), (704, 352)]


def _ft_chunks(lf, nfc):
    return [(i * 128, min(128, lf - i * 128)) for i in range(nfc)]


FT1 = _ft_chunks(LF1, NFC1)
FT2 = _ft_chunks(LF2, NFC2)


# ---------------------------------------------------------------- host consts
def _dft_mats(L, LF):
    t = np.arange(L, dtype=np.float64)[:, None]
    f = np.arange(LF, dtype=np.float64)[None, :]
    ang = 2.0 * np.pi * f * t / L
    rows = 128 * ((L + 127) // 128)
    dc = np.zeros((rows, LF), np.float32)
    ds = np.zeros((rows, LF), np.float32)
    dc[:L] = np.cos(ang)
    ds[:L] = -np.sin(ang)
    return dc, ds


def _idft_mat(L, LF, nfc):
    # rows: chunks 0..nfc-1 = Sre (f), chunks nfc..2nfc-1 = Sim (f); corr scale 1/(L*D)
    t = np.arange(L, dtype=np.float64)[None, :]
    f = np.arange(LF, dtype=np.float64)[:, None]
    ang = 2.0 * np.pi * f * t / L
    w = np.full((LF, 1), 2.0)
    w[0, 0] = 1.0
    w[-1, 0] = 1.0
    scale = 1.0 / (L * D)
    icr = (w * np.cos(ang) * scale).astype(np.float32)
    ism = (-w * np.sin(ang) * scale).astype(np.float32)
    out = np.zeros((2 * nfc * 128, L), np.float32)
    out[:LF] = icr
    out[nfc * 128:nfc * 128 + LF] = ism
    return out


def _chunked_bias(b):
    # [C*128] -> [128, C] per-partition layout
    c = b.shape[0] // 128
    return np.ascontiguousarray(b.reshape(c, 128).T).astype(np.float32)


def host_prep(inp):
    """Return (shared weight/const map, per-core input maps)."""
    g = {}

    def wT(w):  # torch Linear weight [out,in] -> [in,out]
        return np.ascontiguousarray(np.asarray(w).T).astype(np.float32)

    # embeddings: combined [67, 512] (rows j*21+c from tok_W[o,c,j], + time rows)
    for pre, tok, tim in (("e", inp["enc_tok_W"], inp["enc_time_W"]),
                          ("d", inp["dec_tok_W"], inp["dec_time_W"])):
        tok = np.asarray(tok)  # [512, 21, 3]
        m = np.transpose(tok, (2, 1, 0)).reshape(63, D)  # row j*21+c
        t = np.asarray(tim)  # [512, 4]
        g[f"embW_{pre}"] = np.concatenate([m, t.T], 0).astype(np.float32)  # [67,512]

    for l in range(EL):
        for nm in ("q", "k", "v", "o"):
            g[f"eW{nm}{l}"] = wT(inp[f"enc_W{nm}"][l])
            g[f"eb{nm}{l}"] = np.asarray(inp[f"enc_b{nm}"][l]).astype(np.float32)[None, :]
        g[f"eW1{l}"] = wT(inp["enc_W1"][l])
        g[f"eb1{l}"] = _chunked_bias(np.asarray(inp["enc_b1"][l]))
        g[f"eW2{l}"] = wT(inp["enc_W2"][l])
        g[f"eb2{l}"] = _chunked_bias(np.asarray(inp["enc_b2"][l]))
        g[f"ebvC{l}"] = _chunked_bias(np.asarray(inp["enc_bv"][l]))
        g[f"eboC{l}"] = _chunked_bias(np.asarray(inp["enc_bo"][l]))
    for pre in ("ds", "dc"):
        for nm in ("q", "k", "v", "o"):
            g[f"{pre}W{nm}"] = wT(inp[f"{pre}_W{nm}"])
            g[f"{pre}b{nm}"] = np.asarray(inp[f"{pre}_b{nm}"]).astype(np.float32)[None, :]
        g[f"{pre}bvC"] = _chunked_bias(np.asarray(inp[f"{pre}_bv"]))
        g[f"{pre}boC"] = _chunked_bias(np.asarray(inp[f"{pre}_bo"]))
    g["dW1"] = wT(inp["dec_W1"])
    g["db1"] = _chunked_bias(np.asarray(inp["dec_b1"]))
    g["dW2"] = wT(inp["dec_W2"])
    g["db2"] = _chunked_bias(np.asarray(inp["dec_b2"]))

    tw = np.asarray(inp["dec_trend_W"])  # [21, 512, 3]
    g["trendW"] = np.transpose(tw, (2, 1, 0)).reshape(3 * D, CIN).astype(np.float32)
    g["projW"] = wT(inp["dec_proj_W"])  # [512, 21]
    g["projB"] = np.asarray(inp["dec_proj_b"]).astype(np.float32)[:, None]  # [21,1]
    g["enW"] = np.ascontiguousarray(
        np.asarray(inp["enc_norm_w"]).reshape(DC, 128).T).astype(np.float32)
    g["enB"] = np.ascontiguousarray(
        np.asarray(inp["enc_norm_b"]).reshape(DC, 128).T).astype(np.float32)
    g["dnW"] = np.ascontiguousarray(
        np.asarray(inp["dec_norm_w"]).reshape(DC, 128).T).astype(np.float32)
    g["dnB"] = np.ascontiguousarray(
        np.asarray(inp["dec_norm_b"]).reshape(DC, 128).T).astype(np.float32)

    g["ones512"] = np.ones((1, 512), np.float32)
    g["onescol"] = np.ones((128, 1), np.float32)
    g["zeros"] = np.zeros((128, 1056), np.float32)
    g["dftc1"], g["dfts1"] = _dft_mats(L1, LF1)
    g["idft1"] = _idft_mat(L1, LF1, NFC1)
    g["dftc2"], g["dfts2"] = _dft_mats(L2, LF2)
    g["idft2"] = _idft_mat(L2, LF2, NFC2)

    xT = np.transpose(np.asarray(inp["x_enc"]), (0, 2, 1)).astype(np.float32)
    mke = np.transpose(np.asarray(inp["x_mark_enc"]), (0, 2, 1)).astype(np.float32)
    mkd = np.transpose(np.asarray(inp["x_mark_dec"]), (0, 2, 1)).astype(np.float32)
    xT = np.ascontiguousarray(xT)
    mke = np.ascontiguousarray(mke)
    mkd = np.ascontiguousarray(mkd)

    per_core = []
    for c in range(NCORES):
        sl = slice(c * BC, (c + 1) * BC)
        m = dict(g)
        m["xT"] = np.ascontiguousarray(xT[sl])
        m["mkeT"] = np.ascontiguousarray(mke[sl])
        m["mkdT"] = np.ascontiguousarray(mkd[sl])
        per_core.append(m)
    return per_core


# ---------------------------------------------------------------- device build
def build_nc(use_f32r=True, sim=False):
    MMDT = F32R if use_f32r else F32
    GELU = AF.Identity if sim else AF.Gelu
    nc = bacc.Bacc(None, target_bir_lowering=False)

    dram = {}

    def din(name, shape, dt=MMDT):
        dram[name] = nc.dram_tensor(name, list(shape), dt, kind="ExternalInput")
        return dram[name]

    # inputs
    din("xT", (BC, CIN, L1))
    din("mkeT", (BC, 4, L1))
    din("mkdT", (BC, 4, L2))
    din("embW_e", (67, D))
    din("embW_d", (67, D))
    for l in range(EL):
        for nm in ("q", "k", "v", "o"):
            din(f"eW{nm}{l}", (D, D))
            din(f"eb{nm}{l}", (1, D))
        din(f"eW1{l}", (D, DFF))
        din(f"eb1{l}", (128, DFC), F32)
        din(f"eW2{l}", (DFF, D))
        din(f"eb2{l}", (128, DC), F32)
        din(f"ebvC{l}", (128, DC), F32)
        din(f"eboC{l}", (128, DC), F32)
    for pre in ("ds", "dc"):
        for nm in ("q", "k", "v", "o"):
            din(f"{pre}W{nm}", (D, D))
            din(f"{pre}b{nm}", (1, D))
        din(f"{pre}bvC", (128, DC), F32)
        din(f"{pre}boC", (128, DC), F32)
    din("dW1", (D, DFF))
    din("db1", (128, DFC), F32)
    din("dW2", (DFF, D))
    din("db2", (128, DC), F32)
    din("trendW", (3 * D, CIN))
    din("projW", (D, CIN))
    din("projB", (CIN, 1), F32)
    for nm in ("enW", "enB", "dnW", "dnB"):
        din(nm, (128, DC), F32)
    din("ones512", (1, 512))
    din("onescol", (128, 1))
    din("zeros", (128, 1056))
    din("dftc1", (TC1 * 128, LF1))
    din("dfts1", (TC1 * 128, LF1))
    din("idft1", (2 * NFC1 * 128, L1))
    din("dftc2", (TC2 * 128, LF2))
    din("dfts2", (TC2 * 128, LF2))
    din("idft2", (2 * NFC2 * 128, L2))

    out_d = nc.dram_tensor("out", [BC, CIN, PRED], F32, kind="ExternalOutput")

    # DRAM scratch
    enc_dram = nc.dram_tensor("enc_scratch", [BC, DC, 128, L1], MMDT)
    dec_dram = nc.dram_tensor("dec_scratch", [BC, DC, 128, L2], MMDT)
    tsum_dram = nc.dram_tensor("tsum_scratch", [BC, DC, 128, L2], F32)
    seas_dram = nc.dram_tensor("seas_scratch", [BC, CIN, LAB], MMDT)
    trendi_dram = nc.dram_tensor("trendi_scratch", [BC, CIN, L2], F32)

    with TileContext(nc) as tc, ExitStack() as top:
        cpool = top.enter_context(tc.tile_pool(name="consts", bufs=1))
        ones_row = cpool.tile([1, 512], MMDT)
        nc.sync.dma_start(ones_row, dram["ones512"][:])
        ones_col = cpool.tile([128, 1], MMDT)
        nc.sync.dma_start(ones_col, dram["onescol"][:])
        zeros_t = cpool.tile([128, 1056], MMDT)
        nc.sync.dma_start(zeros_t, dram["zeros"][:])

        # ---------------------------------------------------- helper closures
        def linear_T(ps_pool, out, X, W, bias, L, tcn):
            """out[128, tcn, 512] (seq-part) = X.T @ W + bias ; X[128,DC,L]."""
            if L % 128:
                nc.vector.tensor_copy(out[:, tcn - 1, :], zeros_t[:, 0:512])
            for mt in range(tcn):
                m = min(128, L - mt * 128)
                ps = ps_pool.tile([128, 512], F32, tag="mm")
                for kc in range(DC):
                    nc.tensor.matmul(ps[0:m, :], X[:, kc, mt * 128:mt * 128 + m],
                                     W[:, kc, :], start=(kc == 0), stop=False)
                nc.tensor.matmul(ps[0:m, :], ones_row[0:1, 0:m], bias,
                                 start=False, stop=True)
                nc.scalar.copy(out[0:m, mt, :], ps[0:m, :])

        def dft_S(ps_pool, tmp_pool, Sstk, q, k, dftc, dfts, fts, nfc, tck, tcq):
            """Sstk[128, 2*nfc, 1] f32r: stacked sum_c Qf*conj(Kf)."""
            nc.vector.tensor_copy(Sstk[:, :, 0], zeros_t[:, 0:Sstk.shape[1]])
            for ft, (f0, fm) in enumerate(fts):
                scr = {}
                for nm, mat, src, tcs in (("qr", dftc, q, tcq), ("qi", dfts, q, tcq),
                                          ("kr", dftc, k, tck), ("ki", dfts, k, tck)):
                    ps = ps_pool.tile([128, 512], F32, tag="pdft", bufs=2,
                                      name=f"pdft_{nm}")
                    for t in range(tcs):
                        nc.tensor.matmul(ps[0:fm, :], mat[:, t, f0:f0 + fm],
                                         src[:, t, :], start=(t == 0),
                                         stop=(t == tcs - 1))
                    sc = tmp_pool.tile([128, 512], F32, tag=f"s{nm}",
                                       name=f"s{nm}")
                    nc.scalar.copy(sc[0:fm, :], ps[0:fm, :])
                    scr[nm] = sc
                prod = tmp_pool.tile([128, 512], F32, tag="prod", bufs=2)
                cols = tmp_pool.tile([128, 4], F32, tag="cols", bufs=2)
                for ci, (xa, xb) in enumerate((("qr", "kr"), ("qi", "ki"),
                                               ("qi", "kr"), ("qr", "ki"))):
                    nc.vector.scalar_tensor_tensor(
                        prod[0:fm, :], scr[xa][0:fm, :], 1.0, scr[xb][0:fm, :],
                        op0=AL.bypass, op1=AL.mult,
                        accum_out=cols[0:fm, ci:ci + 1])
                nc.vector.tensor_tensor(Sstk[0:fm, ft, 0:1], cols[0:fm, 0:1],
                                        cols[0:fm, 1:2], AL.add)
                nc.vector.tensor_tensor(Sstk[0:fm, nfc + ft, 0:1], cols[0:fm, 2:3],
                                        cols[0:fm, 3:4], AL.subtract)

        def topk_tw(ps_pool, tmp_pool, Sstk, idft, nfc, L, nts):
            """corr -> (twb[128,8] f32, i8[1,8] u32)."""
            corr = tmp_pool.tile([1, L], F32, tag="corr")
            for nt, (n0, n) in enumerate(nts):
                psc = ps_pool.tile([1, 512], F32, tag="corrps")
                for j in range(2 * nfc):
                    nc.tensor.matmul(psc[:, 0:n], Sstk[:, j, 0:1],
                                     idft[:, j, n0:n0 + n],
                                     start=(j == 0), stop=(j == 2 * nfc - 1))
                nc.scalar.copy(corr[:, n0:n0 + n], psc[:, 0:n])
            w8 = tmp_pool.tile([1, 8], F32, tag="w8")
            i8 = tmp_pool.tile([1, 8], U32, tag="i8")
            nc.vector.max_with_indices(w8, i8, corr)
            e6 = tmp_pool.tile([1, 8], F32, tag="e6")
            nc.vector.memset(e6[:, TOPK:8], 0.0)
            nc.vector.tensor_scalar_sub(e6[:, 0:TOPK], w8[:, 0:TOPK], w8[:, 0:1])
            nc.scalar.activation(e6[:, 0:TOPK], e6[:, 0:TOPK], AF.Exp)
            ssum = tmp_pool.tile([1, 1], F32, tag="ssum")
            nc.vector.reduce_sum(ssum, e6[:, 0:TOPK], axis=AX.X)
            nc.vector.reciprocal(ssum, ssum)
            nc.vector.tensor_scalar_mul(e6[:, 0:TOPK], e6[:, 0:TOPK], ssum)
            twb = tmp_pool.tile([128, 8], F32, tag="twb")
            nc.gpsimd.partition_broadcast(twb, e6[0:1, :])
            return twb, i8

        def agg_delays(agg, vv, twb, i8, L):
            for kk in range(TOPK):
                dly = nc.values_load(i8[0:1, kk:kk + 1], min_val=0, max_val=L - 1,
                                     engines=[DVE], skip_runtime_bounds_check=True)
                for mc in range(DC):
                    src = vv[:, mc, bass.ds(dly, L)]
                    if kk == 0:
                        nc.vector.scalar_tensor_tensor(
                            agg[:, mc, :], src, twb[:, 0:1], src,
                            op0=AL.mult, op1=AL.bypass)
                    else:
                        nc.vector.scalar_tensor_tensor(
                            agg[:, mc, :], src, twb[:, kk:kk + 1], agg[:, mc, :],
                            op0=AL.mult, op1=AL.add)

        def out_proj_residual(ps_pool, X, agg, W, boC, nts):
            """X += agg.T@W + bo  (F-layout, in place)."""
            for mc in range(DC):
                for (n0, n) in nts:
                    ps = ps_pool.tile([128, 512], F32, tag="mm")
                    for kc in range(DC):
                        nc.tensor.matmul(ps[:, 0:n], W[:, kc, mc * 128:(mc + 1) * 128],
                                         agg[:, kc, n0:n0 + n],
                                         start=(kc == 0), stop=(kc == DC - 1))
                    nc.vector.scalar_tensor_tensor(
                        X[:, mc, n0:n0 + n], ps[:, 0:n], boC[:, mc:mc + 1],
                        X[:, mc, n0:n0 + n], op0=AL.add, op1=AL.add)

        def decomp(tmp_pool, X, L, chunks=DC, trend_to=None, trend_accum=None,
                   trend_dram_b=None, db=1):
            """X <- X - mavg(X) in place; optionally emit trend (mavg)."""
            for mc in range(chunks):
                xp = tmp_pool.tile([128, L + 2 * PAD], F32, tag="xp", bufs=db)
                nc.scalar.copy(xp[:, PAD:PAD + L], X[:, mc, :])
                nc.vector.tensor_copy(xp[:, 0:PAD],
                                      X[:, mc, 0:1].to_broadcast([128, PAD]))
                nc.vector.tensor_copy(xp[:, PAD + L:],
                                      X[:, mc, L - 1:L].to_broadcast([128, PAD]))
                cs = tmp_pool.tile([128, L + 2 * PAD + 1], F32, tag="cs", bufs=db)
                nc.vector.memset(cs[:, 0:1], 0.0)
                nc.vector.tensor_tensor_scan(cs[:, 1:], xp, xp, 0.0, AL.add, AL.bypass)
                dt = tmp_pool.tile([128, L], F32, tag="dt", bufs=2)
                nc.vector.tensor_tensor(dt, cs[:, MA:MA + L], cs[:, 0:L], AL.subtract)
                if trend_to is not None:
                    nc.vector.tensor_scalar_mul(trend_to[:, mc, :], dt, 1.0 / MA)
                if trend_accum is not None:
                    tt = tmp_pool.tile([128, L], F32, tag="taccum", bufs=2)
                    nc.vector.tensor_scalar_mul(tt, dt, 1.0 / MA)
                    nc.gpsimd.dma_start(tsum_dram[trend_dram_b, mc], tt,
                                        accum_op=(AL.add if trend_accum == "add"
                                                  else AL.bypass))
                nc.vector.scalar_tensor_tensor(X[:, mc, :], dt, -1.0 / MA,
                                               X[:, mc, :], op0=AL.mult, op1=AL.add)

        def ffn(ps_pool, tmp_pool, X, W1, b1C, W2, b2C, L, nts):
            """X += gelu(X@W1+b1)@W2+b2 in place (F-layout)."""
            h = tmp_pool.tile([128, DFC, L], MMDT, tag="h")
            for mh in range(DFC):
                for (n0, n) in nts:
                    ps = ps_pool.tile([128, 512], F32, tag="mm")
                    for kc in range(DC):
                        nc.tensor.matmul(ps[:, 0:n], W1[:, kc, mh * 128:(mh + 1) * 128],
                                         X[:, kc, n0:n0 + n],
                                         start=(kc == 0), stop=(kc == DC - 1))
                    nc.scalar.activation(h[:, mh, n0:n0 + n], ps[:, 0:n], GELU,
                                         bias=b1C[:, mh:mh + 1])
            for mc in range(DC):
                for (n0, n) in nts:
                    ps = ps_pool.tile([128, 512], F32, tag="mm")
                    for kh in range(DFC):
                        nc.tensor.matmul(ps[:, 0:n], W2[:, kh, mc * 128:(mc + 1) * 128],
                                         h[:, kh, n0:n0 + n],
                                         start=(kh == 0), stop=(kh == DFC - 1))
                    nc.vector.scalar_tensor_tensor(
                        X[:, mc, n0:n0 + n], ps[:, 0:n], b2C[:, mc:mc + 1],
                        X[:, mc, n0:n0 + n], op0=AL.add, op1=AL.add)

        def layernorm(ps_pool, tmp_pool, X, Xln, L, nts, wD, bD):
            """Xln = LN(X) over feature dim (partition dim, DC chunks)."""
            xsq = tmp_pool.tile([128, DC, L], MMDT, tag="xsq")
            for mc in range(DC):
                nc.scalar.activation(xsq[:, mc, :], X[:, mc, :], AF.Square)
            pmu = ps_pool.tile([1, len(nts), 512], F32, tag="pmu")
            psq = ps_pool.tile([1, len(nts), 512], F32, tag="psq")
            for nt, (n0, n) in enumerate(nts):
                for kc in range(DC):
                    st, sp = (kc == 0), (kc == DC - 1)
                    nc.tensor.matmul(pmu[:, nt, 0:n], ones_col, X[:, kc, n0:n0 + n],
                                     start=st, stop=sp)
                    nc.tensor.matmul(psq[:, nt, 0:n], ones_col, xsq[:, kc, n0:n0 + n],
                                     start=st, stop=sp)
            stats = tmp_pool.tile([1, 2 * L], F32, tag="stats")
            mu, rstd = stats[:, 0:L], stats[:, L:2 * L]
            for nt, (n0, n) in enumerate(nts):
                nc.vector.tensor_scalar_mul(mu[:, n0:n0 + n], pmu[:, nt, 0:n], 1.0 / D)
                nc.vector.tensor_scalar_mul(rstd[:, n0:n0 + n], psq[:, nt, 0:n], 1.0 / D)
            musq = tmp_pool.tile([1, L], F32, tag="musq")
            nc.vector.tensor_tensor(musq, mu, mu, AL.mult)
            nc.vector.tensor_tensor(rstd, rstd, musq, AL.subtract)
            nc.vector.tensor_scalar_add(rstd, rstd, 1e-5)
            nc.scalar.activation(rstd, rstd, AF.Sqrt)
            nc.vector.reciprocal(rstd, rstd)
            stb = tmp_pool.tile([128, 2 * L], F32, tag="stb")
            nc.gpsimd.partition_broadcast(stb, stats[0:1, :])
            t = tmp_pool.tile([128, L], F32, tag="lnt")
            for mc in range(DC):
                nc.vector.tensor_tensor(t, X[:, mc, :], stb[:, 0:L], AL.subtract)
                nc.vector.tensor_tensor(t, t, stb[:, L:2 * L], AL.mult)
                nc.vector.tensor_scalar_mul(t, t, wD[:, mc:mc + 1])
                nc.vector.tensor_scalar_add(Xln[:, mc, :], t, bD[:, mc:mc + 1])

        # ======================================================== ENCODER
        with tc.tile_pool(name="acts", bufs=1) as apool:
            enc_acts = [apool.tile([128, DC, L1], MMDT, tag=f"enc{b}",
                                   name=f"enc_acts{b}")
                        for b in range(BC)]

            # ---- P0: embedding + init decomposition
            with nc.named_scope("P0_embed"), \
                 tc.tile_pool(name="p0t", bufs=2) as tp, \
                 tc.tile_pool(name="p0w", bufs=1) as wp, \
                 tc.tile_pool(name="p0ps", bufs=4, space="PSUM") as psp:
                embW = wp.tile([67, D], MMDT)
                nc.sync.dma_start(embW, dram["embW_e"][:])
                for b in range(BC):
                    win = tp.tile([67, L1], MMDT, tag="win")
                    nc.sync.dma_start(win[0:CIN, 1:L1], dram["xT"][b, :, 0:L1 - 1])
                    nc.sync.dma_start(win[0:CIN, 0:1], dram["xT"][b, :, L1 - 1:L1])
                    nc.sync.dma_start(win[CIN:2 * CIN, :], dram["xT"][b])
                    nc.sync.dma_start(win[2 * CIN:3 * CIN, 0:L1 - 1],
                                      dram["xT"][b, :, 1:L1])
                    nc.sync.dma_start(win[2 * CIN:3 * CIN, L1 - 1:L1],
                                      dram["xT"][b, :, 0:1])
                    nc.sync.dma_start(win[63:67, :], dram["mkeT"][b])
                    for mc in range(DC):
                        for (n0, n) in NT1:
                            ps = psp.tile([128, 512], F32, tag="mm")
                            nc.tensor.matmul(ps[:, 0:n],
                                             embW[:, mc * 128:(mc + 1) * 128],
                                             win[:, n0:n0 + n], start=True, stop=True)
                            nc.scalar.copy(enc_acts[b][:, mc, n0:n0 + n], ps[:, 0:n])

                # init series_decomp of x_enc (packed [84, .])
                xe = tp.tile([84, L1], MMDT, tag="xe")
                for b in range(BC):
                    nc.sync.dma_start(xe[b * CIN:(b + 1) * CIN, :], dram["xT"][b])
                xp = tp.tile([84, L1 + 2 * PAD], F32, tag="ixp")
                nc.scalar.copy(xp[:, PAD:PAD + L1], xe)
                nc.vector.tensor_copy(xp[:, 0:PAD], xe[:, 0:1].to_broadcast([84, PAD]))
                nc.vector.tensor_copy(xp[:, PAD + L1:],
                                      xe[:, L1 - 1:L1].to_broadcast([84, PAD]))
                cs = tp.tile([84, L1 + 2 * PAD + 1], F32, tag="ics")
                nc.vector.memset(cs[:, 0:1], 0.0)
                nc.vector.tensor_tensor_scan(cs[:, 1:], xp, xp, 0.0, AL.add, AL.bypass)
                dt = tp.tile([84, L1], F32, tag="idt")
                nc.vector.tensor_tensor(dt, cs[:, MA:MA + L1], cs[:, 0:L1], AL.subtract)
                seas = tp.tile([84, L1], MMDT, tag="iseas")
                nc.vector.scalar_tensor_tensor(seas, dt, -1.0 / MA, xe,
                                               op0=AL.mult, op1=AL.add)
                trend = tp.tile([84, L1], F32, tag="itrend")
                nc.vector.tensor_scalar_mul(trend, dt, 1.0 / MA)
                mean = tp.tile([84, 1], F32, tag="imean")
                nc.vector.reduce_sum(mean, xe, axis=AX.X)
                nc.vector.tensor_scalar_mul(mean, mean, 1.0 / L1)
                meanb = tp.tile([84, PRED], F32, tag="imeanb")
                nc.vector.tensor_copy(meanb, mean.to_broadcast([84, PRED]))
                for b in range(BC):
                    sl = slice(b * CIN, (b + 1) * CIN)
                    nc.sync.dma_start(seas_dram[b], seas[sl, L1 - LAB:L1])
                    nc.sync.dma_start(trendi_dram[b, :, 0:LAB], trend[sl, L1 - LAB:L1])
                    nc.sync.dma_start(trendi_dram[b, :, LAB:L2], meanb[sl, :])

            # ---- P1/P2: encoder layers
            if True:
                for l in range(EL):
                    with nc.named_scope(f"enc{l}_att"), \
                         tc.tile_pool(name="dft1", bufs=1) as dft1p, \
                         tc.tile_pool(name="eatw", bufs=1) as wp, \
                         tc.tile_pool(name="eatt", bufs=1) as tp, \
                         tc.tile_pool(name="eatps", bufs=1, space="PSUM") as psp, \
                         tc.tile_pool(name="eatps2", bufs=3, space="PSUM") as psp2:
                        dftc1 = dft1p.tile([128, TC1, LF1], MMDT, tag="dftc1")
                        dfts1 = dft1p.tile([128, TC1, LF1], MMDT, tag="dfts1")
                        idft1 = dft1p.tile([128, 2 * NFC1, L1], MMDT, tag="idft1")
                        nc.sync.dma_start(dftc1, dram["dftc1"][:].rearrange(
                            "(c p) f -> p c f", p=128))
                        nc.sync.dma_start(dfts1, dram["dfts1"][:].rearrange(
                            "(c p) f -> p c f", p=128))
                        nc.sync.dma_start(idft1, dram["idft1"][:].rearrange(
                            "(c p) f -> p c f", p=128))
                        Ws = {}
                        for nm in ("q", "k", "v", "o"):
                            Ws[nm] = wp.tile([128, DC, D], MMDT, tag=f"W{nm}",
                                             name=f"W{nm}")
                            nc.sync.dma_start(Ws[nm], dram[f"eW{nm}{l}"][:].rearrange(
                                "(c p) f -> p c f", p=128))
                        bq = wp.tile([1, D], MMDT, tag="bq")
                        bk = wp.tile([1, D], MMDT, tag="bk")
                        nc.sync.dma_start(bq, dram[f"ebq{l}"][:])
                        nc.sync.dma_start(bk, dram[f"ebk{l}"][:])
                        bvC = wp.tile([128, DC], F32, tag="bvC")
                        boC = wp.tile([128, DC], F32, tag="boC")
                        nc.sync.dma_start(bvC, dram[f"ebvC{l}"][:])
                        nc.sync.dma_start(boC, dram[f"eboC{l}"][:])
                        for b in range(BC):
                            X = enc_acts[b]
                            q = tp.tile([128, TC1, 512], MMDT, tag="q")
                            k = tp.tile([128, TC1, 512], MMDT, tag="k", bufs=2)
                            linear_T(psp2, q, X, Ws["q"], bq, L1, TC1)
                            linear_T(psp2, k, X, Ws["k"], bk, L1, TC1)
                            Sstk = tp.tile([128, 2 * NFC1, 1], MMDT, tag="Sstk")
                            dft_S(psp, tp, Sstk, q, k, dftc1, dfts1, FT1, NFC1,
                                  TC1, TC1)
                            twb, i8 = topk_tw(psp, tp, Sstk, idft1, NFC1, L1, NT1)
                            vv = tp.tile([128, DC, 2 * L1], F32, tag="q")
                            for mc in range(DC):
                                for (n0, n) in NT1:
                                    ps = psp2.tile([128, 512], F32, tag="mm")
                                    for kc in range(DC):
                                        nc.tensor.matmul(
                                            ps[:, 0:n],
                                            Ws["v"][:, kc, mc * 128:(mc + 1) * 128],
                                            X[:, kc, n0:n0 + n],
                                            start=(kc == 0), stop=(kc == DC - 1))
                                    nc.scalar.activation(vv[:, mc, n0:n0 + n],
                                                         ps[:, 0:n], AF.Identity,
                                                         bias=bvC[:, mc:mc + 1])
                                    nc.scalar.activation(vv[:, mc, L1 + n0:L1 + n0 + n],
                                                         ps[:, 0:n], AF.Identity,
                                                         bias=bvC[:, mc:mc + 1])
                            agg = tp.tile([128, DC, L1], MMDT, tag="k", bufs=2)
                            agg_delays(agg, vv, twb, i8, L1)
                            out_proj_residual(psp2, X, agg, Ws["o"], boC, NT1)
                            decomp(tp, X, L1)

                    with nc.named_scope(f"enc{l}_ffn"), \
                         tc.tile_pool(name="effw", bufs=1) as wp, \
                         tc.tile_pool(name="efft", bufs=1) as tp, \
                         tc.tile_pool(name="effps", bufs=6, space="PSUM") as psp:
                        W1 = wp.tile([128, DC, DFF], MMDT, tag="W1")
                        W2 = wp.tile([128, DFC, D], MMDT, tag="W2")
                        nc.sync.dma_start(W1, dram[f"eW1{l}"][:].rearrange(
                            "(c p) f -> p c f", p=128))
                        nc.sync.dma_start(W2, dram[f"eW2{l}"][:].rearrange(
                            "(c p) f -> p c f", p=128))
                        b1C = wp.tile([128, DFC], F32, tag="b1C")
                        b2C = wp.tile([128, DC], F32, tag="b2C")
                        nc.sync.dma_start(b1C, dram[f"eb1{l}"][:])
                        nc.sync.dma_start(b2C, dram[f"eb2{l}"][:])
                        for b in range(BC):
                            ffn(psp, tp, enc_acts[b], W1, b1C, W2, b2C, L1, NT1)
                            decomp(tp, enc_acts[b], L1, db=2)

            # ---- P3: final encoder LN -> enc_dram
            with nc.named_scope("enc_ln"), \
                 tc.tile_pool(name="lnt", bufs=1) as tp, \
                 tc.tile_pool(name="lnw", bufs=1) as wp, \
                 tc.tile_pool(name="lnps", bufs=1, space="PSUM") as psp:
                enW = wp.tile([128, DC], F32, tag="enW")
                enB = wp.tile([128, DC], F32, tag="enB")
                nc.sync.dma_start(enW, dram["enW"][:])
                nc.sync.dma_start(enB, dram["enB"][:])
                for b in range(BC):
                    xln = tp.tile([128, DC, L1], MMDT, tag="xln")
                    layernorm(psp, tp, enc_acts[b], xln, L1, NT1, enW, enB)
                    for mc in range(DC):
                        nc.sync.dma_start(enc_dram[b, mc], xln[:, mc, :])

        # ======================================================== DECODER
        # ---- P4: decoder embedding -> dec_dram
        with nc.named_scope("dec_embed"), \
             tc.tile_pool(name="p4t", bufs=2) as tp, \
             tc.tile_pool(name="p4w", bufs=1) as wp, \
             tc.tile_pool(name="p4ps", bufs=4, space="PSUM") as psp:
            embW = wp.tile([67, D], MMDT)
            nc.sync.dma_start(embW, dram["embW_d"][:])
            for b in range(BC):
                win = tp.tile([67, L2], MMDT, tag="win2")
                nc.vector.tensor_copy(win[0:63, :], zeros_t[0:63, 0:L2])
                nc.sync.dma_start(win[0:CIN, 1:LAB + 1], seas_dram[b])
                nc.sync.dma_start(win[CIN:2 * CIN, 0:LAB], seas_dram[b])
                nc.sync.dma_start(win[2 * CIN:3 * CIN, 0:LAB - 1],
                                  seas_dram[b, :, 1:LAB])
                nc.sync.dma_start(win[2 * CIN:3 * CIN, L2 - 1:L2],
                                  seas_dram[b, :, 0:1])
                nc.sync.dma_start(win[63:67, :], dram["mkdT"][b])
                for mc in range(DC):
                    for (n0, n) in NT2:
                        ps = psp.tile([128, 512], F32, tag="mm")
                        nc.tensor.matmul(ps[:, 0:n], embW[:, mc * 128:(mc + 1) * 128],
                                         win[:, n0:n0 + n], start=True, stop=True)
                        xpart = tp.tile([128, 512], MMDT, tag="xpart")
                        nc.scalar.copy(xpart[:, 0:n], ps[:, 0:n])
                        nc.sync.dma_start(dec_dram[b, mc, :, n0:n0 + n], xpart[:, 0:n])

        # ---- P5/P6: decoder attentions
        for phase, pre in (("self", "ds"), ("cross", "dc")):
            with nc.named_scope(f"dec_{phase}"), \
                 tc.tile_pool(name="datw", bufs=1) as wp, \
                 tc.tile_pool(name="datt", bufs=1) as tp, \
                 tc.tile_pool(name="dft2", bufs=1) as dp, \
                 tc.tile_pool(name="datps", bufs=1, space="PSUM") as psp, \
                 tc.tile_pool(name="datps2", bufs=3, space="PSUM") as psp2:
                Ws = {}
                for nm in ("q", "k", "v", "o"):
                    Ws[nm] = wp.tile([128, DC, D], MMDT, tag=f"W{nm}",
                                     name=f"W{nm}")
                    nc.sync.dma_start(Ws[nm], dram[f"{pre}W{nm}"][:].rearrange(
                        "(c p) f -> p c f", p=128))
                bq = wp.tile([1, D], MMDT, tag="bq")
                bk = wp.tile([1, D], MMDT, tag="bk")
                nc.sync.dma_start(bq, dram[f"{pre}bq"][:])
                nc.sync.dma_start(bk, dram[f"{pre}bk"][:])
                bvC = wp.tile([128, DC], F32, tag="bvC")
                boC = wp.tile([128, DC], F32, tag="boC")
                nc.sync.dma_start(bvC, dram[f"{pre}bvC"][:])
                nc.sync.dma_start(boC, dram[f"{pre}boC"][:])
                kvsrc_chunks = TC2 if phase == "self" else TC1
                for b in range(BC):
                    X = tp.tile([128, DC, L2], MMDT, tag="Xd")
                    for mc in range(DC):
                        nc.sync.dma_start(X[:, mc, :], dec_dram[b, mc])
                    if phase == "self":
                        KV = X
                    else:
                        KV = tp.tile([128, DC, L1], MMDT, tag="KV")
                        for mc in range(DC):
                            nc.sync.dma_start(KV[:, mc, :], enc_dram[b, mc])
                    q = tp.tile([128, TC2, 512], MMDT, tag="q2")
                    k = tp.tile([128, TC2, 512], MMDT, tag="k2")
                    linear_T(psp2, q, X, Ws["q"], bq, L2, TC2)
                    linear_T(psp2, k, KV, Ws["k"], bk,
                             L2 if phase == "self" else L1, kvsrc_chunks)
                    dftcs = dp.tile([128, TC2, 2 * LF2], MMDT, tag="dftbig")
                    nc.sync.dma_start(dftcs[:, :, 0:LF2], dram["dftc2"][:].rearrange(
                        "(c p) f -> p c f", p=128))
                    nc.sync.dma_start(dftcs[:, :, LF2:], dram["dfts2"][:].rearrange(
                        "(c p) f -> p c f", p=128))
                    Sstk = tp.tile([128, 2 * NFC2, 1], MMDT, tag="Sstk2")
                    dft_S(psp, tp, Sstk, q, k, dftcs[:, :, 0:LF2],
                          dftcs[:, :, LF2:2 * LF2], FT2, NFC2, kvsrc_chunks, TC2)
                    idft2 = dp.tile([128, 2 * NFC2, L2], MMDT, tag="dftbig")
                    nc.sync.dma_start(idft2, dram["idft2"][:].rearrange(
                        "(c p) f -> p c f", p=128))
                    twb, i8 = topk_tw(psp, tp, Sstk, idft2, NFC2, L2, NT2)
                    VL = 2 * L2
                    vv = tp.tile([128, DC, VL], F32, tag="q2")
                    if phase == "cross":
                        nc.vector.memset(vv, 0.0)
                    kvL = L2 if phase == "self" else L1
                    kvNT = NT2 if phase == "self" else NT1
                    for mc in range(DC):
                        for (n0, n) in kvNT:
                            ps = psp2.tile([128, 512], F32, tag="mm")
                            for kc in range(DC):
                                nc.tensor.matmul(
                                    ps[:, 0:n],
                                    Ws["v"][:, kc, mc * 128:(mc + 1) * 128],
                                    KV[:, kc, n0:n0 + n],
                                    start=(kc == 0), stop=(kc == DC - 1))
                            nc.scalar.activation(vv[:, mc, n0:n0 + n], ps[:, 0:n],
                                                 AF.Identity, bias=bvC[:, mc:mc + 1])
                            nc.scalar.activation(vv[:, mc, L2 + n0:L2 + n0 + n],
                                                 ps[:, 0:n], AF.Identity,
                                                 bias=bvC[:, mc:mc + 1])
                    agg = tp.tile([128, DC, L2], MMDT, tag="k2")
                    agg_delays(agg, vv, twb, i8, L2)
                    out_proj_residual(psp2, X, agg, Ws["o"], boC, NT2)
                    decomp(tp, X, L2, trend_accum=("bypass" if phase == "self"
                                                   else "add"), trend_dram_b=b)
                    for mc in range(DC):
                        nc.sync.dma_start(dec_dram[b, mc], X[:, mc, :])

        # ---- P7: decoder FFN
        with nc.named_scope("dec_ffn"), \
             tc.tile_pool(name="dffw", bufs=1) as wp, \
             tc.tile_pool(name="dfft", bufs=1) as tp, \
             tc.tile_pool(name="dffps", bufs=6, space="PSUM") as psp:
            W1 = wp.tile([128, DC, DFF], MMDT, tag="W1")
            W2 = wp.tile([128, DFC, D], MMDT, tag="W2")
            nc.sync.dma_start(W1, dram["dW1"][:].rearrange("(c p) f -> p c f", p=128))
            nc.sync.dma_start(W2, dram["dW2"][:].rearrange("(c p) f -> p c f", p=128))
            b1C = wp.tile([128, DFC], F32, tag="b1C")
            b2C = wp.tile([128, DC], F32, tag="b2C")
            nc.sync.dma_start(b1C, dram["db1"][:])
            nc.sync.dma_start(b2C, dram["db2"][:])
            for b in range(BC):
                X = tp.tile([128, DC, L2], MMDT, tag="Xd")
                for mc in range(DC):
                    nc.sync.dma_start(X[:, mc, :], dec_dram[b, mc])
                ffn(psp, tp, X, W1, b1C, W2, b2C, L2, NT2)
                decomp(tp, X, L2, trend_accum="add", trend_dram_b=b, db=2)
                for mc in range(DC):
                    nc.sync.dma_start(dec_dram[b, mc], X[:, mc, :])

        # ---- P8: final (LN + proj, trend conv, combine)
        with nc.named_scope("final"), \
             tc.tile_pool(name="fint", bufs=1) as tp, \
             tc.tile_pool(name="finw", bufs=1) as wp, \
             tc.tile_pool(name="finps", bufs=1, space="PSUM") as psp, \
             tc.tile_pool(name="finps2", bufs=2, space="PSUM") as psp2:
            trendW = wp.tile([128, 3 * DC, CIN], MMDT, tag="trendW")
            nc.sync.dma_start(trendW, dram["trendW"][:].rearrange(
                "(c p) f -> p c f", p=128))
            projW = wp.tile([128, DC, CIN], MMDT, tag="projW")
            nc.sync.dma_start(projW, dram["projW"][:].rearrange(
                "(c p) f -> p c f", p=128))
            projB = wp.tile([CIN, 1], F32, tag="projB")
            nc.sync.dma_start(projB, dram["projB"][:])
            dnW = wp.tile([128, DC], F32, tag="dnW")
            dnB = wp.tile([128, DC], F32, tag="dnB")
            nc.sync.dma_start(dnW, dram["dnW"][:])
            nc.sync.dma_start(dnB, dram["dnB"][:])
            for b in range(BC):
                X = tp.tile([128, DC, L2], MMDT, tag="Xd")
                for mc in range(DC):
                    nc.sync.dma_start(X[:, mc, :], dec_dram[b, mc])
                xln = tp.tile([128, DC, L2], MMDT, tag="xln2")
                layernorm(psp, tp, X, xln, L2, NT2, dnW, dnB)
                seasonal = tp.tile([CIN, L2], F32, tag="seasonal")
                for (n0, n) in NT2:
                    ps = psp2.tile([CIN, 512], F32, tag="sm")
                    for kc in range(DC):
                        nc.tensor.matmul(ps[:, 0:n], projW[:, kc, :],
                                         xln[:, kc, n0:n0 + n],
                                         start=(kc == 0), stop=(kc == DC - 1))
                    nc.scalar.activation(seasonal[:, n0:n0 + n], ps[:, 0:n],
                                         AF.Identity, bias=projB)
                # trend conv windows [12 chunks, L2] circular
                tsum = tp.tile([128, DC, L2], F32, tag="tsum")
                for mc in range(DC):
                    nc.sync.dma_start(tsum[:, mc, :], tsum_dram[b, mc])
                winT = tp.tile([128, 3 * DC, L2], MMDT, tag="winT")
                for mc in range(DC):
                    # j=0: shift -1 ; j=1: center ; j=2: shift +1 (circular)
                    nc.scalar.copy(winT[:, mc, 1:L2], tsum[:, mc, 0:L2 - 1])
                    nc.scalar.copy(winT[:, mc, 0:1], tsum[:, mc, L2 - 1:L2])
                    nc.scalar.copy(winT[:, DC + mc, :], tsum[:, mc, :])
                    nc.scalar.copy(winT[:, 2 * DC + mc, 0:L2 - 1], tsum[:, mc, 1:L2])
                    nc.scalar.copy(winT[:, 2 * DC + mc, L2 - 1:L2], tsum[:, mc, 0:1])
                trendi = tp.tile([CIN, L2], F32, tag="trendi")
                nc.sync.dma_start(trendi, trendi_dram[b])
                outt = tp.tile([CIN, PRED], F32, tag="outt")
                for nt, (n0, n) in enumerate(NT2):
                    ps = psp2.tile([CIN, 512], F32, tag="sm")
                    for j in range(3 * DC):
                        nc.tensor.matmul(ps[:, 0:n], trendW[:, j, :],
                                         winT[:, j, n0:n0 + n],
                                         start=(j == 0), stop=(j == 3 * DC - 1))
                    trend_sl = tp.tile([CIN, 512], F32, tag="trend_sl")
                    nc.vector.tensor_tensor(trend_sl[:, 0:n], ps[:, 0:n],
                                            trendi[:, n0:n0 + n], AL.add)
                    # add seasonal, write PRED slice (cols >= LAB)
                    lo = max(n0, LAB)
                    hi = n0 + n
                    if hi > lo:
                        nc.vector.tensor_tensor(
                            outt[:, lo - LAB:hi - LAB], trend_sl[:, lo - n0:hi - n0],
                            seasonal[:, lo:hi], AL.add)
                nc.sync.dma_start(out_d[b], outt)

    nc.compile()
    return nc


# ---------------------------------------------------------------- entry point
_CACHE = {}
LAST_EXEC_NS = [None]
SHARDED_INPUTS = ("xT", "mkeT", "mkdT")


def _get_executor(use_f32r=True):
    if "run" in _CACHE:
        return _CACHE["run"]
    import jax
    from jax.sharding import Mesh, PartitionSpec
    try:
        from jax.experimental.shard_map import shard_map
    except Exception:
        from jax.shard_map import shard_map
    from concourse import bass2jax

    bass2jax.install_neuronx_cc_hook()
    nc = build_nc(use_f32r=use_f32r)
    _CACHE["nc"] = nc

    part_name = (nc.partition_id_tensor.name
                 if nc.partition_id_tensor else None)
    in_names, out_names, out_avals = [], [], []
    for alloc in nc.m.functions[0].allocations:
        if not isinstance(alloc, mybir.MemoryLocationSet):
            continue
        name = alloc.memorylocations[0].name
        if alloc.kind == "ExternalInput":
            if name != part_name:
                in_names.append(name)
        elif alloc.kind == "ExternalOutput":
            out_names.append(name)
            out_avals.append(jax.core.ShapedArray(
                tuple(alloc.tensor_shape), mybir.dt.np(alloc.dtype)))
    n_params = len(in_names)
    n_outs = len(out_names)
    all_names = list(in_names) + list(out_names)
    if part_name is not None:
        all_names.append(part_name)

    donate = tuple(range(n_params, n_params + n_outs))

    def _body(*args):
        operands = list(args)
        if part_name is not None:
            operands.append(bass2jax.partition_id_tensor())
        outs = bass2jax._bass_exec_p.bind(
            *operands,
            out_avals=tuple(out_avals),
            in_names=tuple(all_names),
            out_names=tuple(out_names),
            lowering_input_output_aliases=(),
            sim_require_finite=True,
            sim_require_nnan=True,
            nc=nc,
        )
        return tuple(outs)

    devices = [d for d in jax.devices() if d.platform != "cpu"][:NCORES]
    if len(devices) < NCORES:
        devices = jax.devices()[:NCORES]
    assert len(devices) == NCORES, f"need {NCORES} neuron cores"
    mesh = Mesh(np.asarray(devices), ("core",))

    def spec(name):
        return (PartitionSpec("core") if name in SHARDED_INPUTS
                else PartitionSpec())

    in_specs = tuple(spec(n) for n in in_names) + \
        (PartitionSpec("core"),) * n_outs
    out_specs = (PartitionSpec("core"),) * n_outs
    sharded = jax.jit(
        shard_map(_body, mesh=mesh, in_specs=in_specs, out_specs=out_specs,
                  check_rep=False),
        donate_argnums=donate, keep_unused=True)

    def run(per_core_maps):
        args = []
        for name in in_names:
            if name in SHARDED_INPUTS:
                args.append(np.concatenate(
                    [m[name] for m in per_core_maps], 0))
            else:
                args.append(np.asarray(per_core_maps[0][name]))
        zeros = [np.zeros((NCORES * a.shape[0], *a.shape[1:]), a.dtype)
                 for a in (np.zeros(s.shape, s.dtype) for s in out_avals)]
        outs = sharded(*args, *zeros)
        res = []
        for c in range(NCORES):
            res.append({name: np.asarray(outs[i]).reshape(
                NCORES, *out_avals[i].shape)[c]
                for i, name in enumerate(out_names)})
        return res

    _CACHE["run"] = run
    _CACHE["sharded"] = sharded
    _CACHE["in_names"] = in_names
    _CACHE["out_avals"] = out_avals
    return run


def kernel(**inputs):
    use_f32r = os.environ.get("AK_F32", "") != "1"
    per_core = host_prep(inputs)
    run = _get_executor(use_f32r=use_f32r)
    results = run(per_core)
    outs = [np.transpose(r["out"], (0, 2, 1)) for r in results]
    return np.concatenate(outs, 0).astype(np.float32)


if __name__ == "__main__":
    pass


# revision 42
# speedup vs baseline: 31.2083x; 1.0024x over previous
"""Autoformer forward on 8 Trainium2 NeuronCores, data-parallel over batch."""
import math
import os
import sys

sys.path.insert(0, "/opt/trn_rl_repo")
import numpy as np
from contextlib import ExitStack

import concourse.bass as bass
import concourse.bacc as bacc
import concourse.mybir as mybir
from concourse.tile import TileContext
from concourse.bass_utils import run_bass_kernel_spmd

AL = mybir.AluOpType
AF = mybir.ActivationFunctionType
F32 = mybir.dt.float32
F32R = mybir.dt.float32r
U32 = mybir.dt.uint32
AX = mybir.AxisListType
DVE = mybir.EngineType.DVE

B, SEQ, LAB, PRED = 32, 720, 336, 720
CIN, D, H, DFF, EL, MA = 21, 512, 8, 2048, 2, 25
L1, L2 = SEQ, LAB + PRED            # 720, 1056
LF1, LF2 = L1 // 2 + 1, L2 // 2 + 1  # 361, 529
TOPK = 6
PAD = (MA - 1) // 2                 # 12
NCORES = 8
BC = B // NCORES                    # 4 batches per core
DC = D // 128                       # 4 feature chunks
DFC = DFF // 128                    # 16

TC1 = (L1 + 127) // 128             # 6   seq chunks (enc)
TC2 = (L2 + 127) // 128             # 9   seq chunks (dec)
NFC1 = (LF1 + 127) // 128           # 3   freq chunks (enc)
NFC2 = (LF2 + 127) // 128           # 5   freq chunks (dec)
NT1 = [(0, 360), (360, 360)]
NT2 = [(0, 352), (352, 352), (704, 352)]


def _ft_chunks(lf, nfc):
    return [(i * 128, min(128, lf - i * 128)) for i in range(nfc)]


FT1 = _ft_chunks(LF1, NFC1)
FT2 = _ft_chunks(LF2, NFC2)


# ---------------------------------------------------------------- host consts
def _dft_mats(L, LF):
    t = np.arange(L, dtype=np.float64)[:, None]
    f = np.arange(LF, dtype=np.float64)[None, :]
    ang = 2.0 * np.pi * f * t / L
    rows = 128 * ((L + 127) // 128)
    dc = np.zeros((rows, LF), np.float32)
    ds = np.zeros((rows, LF), np.float32)
    dc[:L] = np.cos(ang)
    ds[:L] = -np.sin(ang)
    return dc, ds


def _idft_mat(L, LF, nfc):
    # rows: chunks 0..nfc-1 = Sre (f), chunks nfc..2nfc-1 = Sim (f); corr scale 1/(L*D)
    t = np.arange(L, dtype=np.float64)[None, :]
    f = np.arange(LF, dtype=np.float64)[:, None]
    ang = 2.0 * np.pi * f * t / L
    w = np.full((LF, 1), 2.0)
    w[0, 0] = 1.0
    w[-1, 0] = 1.0
    scale = 1.0 / (L * D)
    icr = (w * np.cos(ang) * scale).astype(np.float32)
    ism = (-w * np.sin(ang) * scale).astype(np.float32)
    out = np.zeros((2 * nfc * 128, L), np.float32)
    out[:LF] = icr
    out[nfc * 128:nfc * 128 + LF] = ism
    return out


def _chunked_bias(b):
    # [C*128] -> [128, C] per-partition layout
    c = b.shape[0] // 128
    return np.ascontiguousarray(b.reshape(c, 128).T).astype(np.float32)


def host_prep(inp):
    """Return (shared weight/const map, per-core input maps)."""
    g = {}

    def wT(w):  # torch Linear weight [out,in] -> [in,out]
        return np.ascontiguousarray(np.asarray(w).T).astype(np.float32)

    # embeddings: combined [67, 512] (rows j*21+c from tok_W[o,c,j], + time rows)
    for pre, tok, tim in (("e", inp["enc_tok_W"], inp["enc_time_W"]),
                          ("d", inp["dec_tok_W"], inp["dec_time_W"])):
        tok = np.asarray(tok)  # [512, 21, 3]
        m = np.transpose(tok, (2, 1, 0)).reshape(63, D)  # row j*21+c
        t = np.asarray(tim)  # [512, 4]
        g[f"embW_{pre}"] = np.concatenate([m, t.T], 0).astype(np.float32)  # [67,512]

    for l in range(EL):
        for nm in ("q", "k", "v", "o"):
            g[f"eW{nm}{l}"] = wT(inp[f"enc_W{nm}"][l])
            g[f"eb{nm}{l}"] = np.asarray(inp[f"enc_b{nm}"][l]).astype(np.float32)[None, :]
        g[f"eW1{l}"] = wT(inp["enc_W1"][l])
        g[f"eb1{l}"] = _chunked_bias(np.asarray(inp["enc_b1"][l]))
        g[f"eW2{l}"] = wT(inp["enc_W2"][l])
        g[f"eb2{l}"] = _chunked_bias(np.asarray(inp["enc_b2"][l]))
        g[f"ebvC{l}"] = _chunked_bias(np.asarray(inp["enc_bv"][l]))
        g[f"eboC{l}"] = _chunked_bias(np.asarray(inp["enc_bo"][l]))
    for pre in ("ds", "dc"):
        for nm in ("q", "k", "v", "o"):
            g[f"{pre}W{nm}"] = wT(inp[f"{pre}_W{nm}"])
            g[f"{pre}b{nm}"] = np.asarray(inp[f"{pre}_b{nm}"]).astype(np.float32)[None, :]
        g[f"{pre}bvC"] = _chunked_bias(np.asarray(inp[f"{pre}_bv"]))
        g[f"{pre}boC"] = _chunked_bias(np.asarray(inp[f"{pre}_bo"]))
    g["dW1"] = wT(inp["dec_W1"])
    g["db1"] = _chunked_bias(np.asarray(inp["dec_b1"]))
    g["dW2"] = wT(inp["dec_W2"])
    g["db2"] = _chunked_bias(np.asarray(inp["dec_b2"]))

    tw = np.asarray(inp["dec_trend_W"])  # [21, 512, 3]
    g["trendW"] = np.transpose(tw, (2, 1, 0)).reshape(3 * D, CIN).astype(np.float32)
    g["projW"] = wT(inp["dec_proj_W"])  # [512, 21]
    g["projB"] = np.asarray(inp["dec_proj_b"]).astype(np.float32)[:, None]  # [21,1]
    g["enW"] = np.ascontiguousarray(
        np.asarray(inp["enc_norm_w"]).reshape(DC, 128).T).astype(np.float32)
    g["enB"] = np.ascontiguousarray(
        np.asarray(inp["enc_norm_b"]).reshape(DC, 128).T).astype(np.float32)
    g["dnW"] = np.ascontiguousarray(
        np.asarray(inp["dec_norm_w"]).reshape(DC, 128).T).astype(np.float32)
    g["dnB"] = np.ascontiguousarray(
        np.asarray(inp["dec_norm_b"]).reshape(DC, 128).T).astype(np.float32)

    g["ones512"] = np.ones((1, 512), np.float32)
    g["onescol"] = np.ones((128, 1), np.float32)
    g["zeros"] = np.zeros((128, 1056), np.float32)
    g["dftc1"], g["dfts1"] = _dft_mats(L1, LF1)
    g["idft1"] = _idft_mat(L1, LF1, NFC1)
    g["dftc2"], g["dfts2"] = _dft_mats(L2, LF2)
    g["idft2"] = _idft_mat(L2, LF2, NFC2)

    xT = np.transpose(np.asarray(inp["x_enc"]), (0, 2, 1)).astype(np.float32)
    mke = np.transpose(np.asarray(inp["x_mark_enc"]), (0, 2, 1)).astype(np.float32)
    mkd = np.transpose(np.asarray(inp["x_mark_dec"]), (0, 2, 1)).astype(np.float32)
    xT = np.ascontiguousarray(xT)
    mke = np.ascontiguousarray(mke)
    mkd = np.ascontiguousarray(mkd)

    per_core = []
    for c in range(NCORES):
        sl = slice(c * BC, (c + 1) * BC)
        m = dict(g)
        m["xT"] = np.ascontiguousarray(xT[sl])
        m["mkeT"] = np.ascontiguousarray(mke[sl])
        m["mkdT"] = np.ascontiguousarray(mkd[sl])
        per_core.append(m)
    return per_core


# ---------------------------------------------------------------- device build
def build_nc(use_f32r=True, sim=False):
    MMDT = F32R if use_f32r else F32
    GELU = AF.Identity if sim else AF.Gelu
    nc = bacc.Bacc(None, target_bir_lowering=False)

    dram = {}

    def din(name, shape, dt=MMDT):
        dram[name] = nc.dram_tensor(name, list(shape), dt, kind="ExternalInput")
        return dram[name]

    # inputs
    din("xT", (BC, CIN, L1))
    din("mkeT", (BC, 4, L1))
    din("mkdT", (BC, 4, L2))
    din("embW_e", (67, D))
    din("embW_d", (67, D))
    for l in range(EL):
        for nm in ("q", "k", "v", "o"):
            din(f"eW{nm}{l}", (D, D))
            din(f"eb{nm}{l}", (1, D))
        din(f"eW1{l}", (D, DFF))
        din(f"eb1{l}", (128, DFC), F32)
        din(f"eW2{l}", (DFF, D))
        din(f"eb2{l}", (128, DC), F32)
        din(f"ebvC{l}", (128, DC), F32)
        din(f"eboC{l}", (128, DC), F32)
    for pre in ("ds", "dc"):
        for nm in ("q", "k", "v", "o"):
            din(f"{pre}W{nm}", (D, D))
            din(f"{pre}b{nm}", (1, D))
        din(f"{pre}bvC", (128, DC), F32)
        din(f"{pre}boC", (128, DC), F32)
    din("dW1", (D, DFF))
    din("db1", (128, DFC), F32)
    din("dW2", (DFF, D))
    din("db2", (128, DC), F32)
    din("trendW", (3 * D, CIN))
    din("projW", (D, CIN))
    din("projB", (CIN, 1), F32)
    for nm in ("enW", "enB", "dnW", "dnB"):
        din(nm, (128, DC), F32)
    din("ones512", (1, 512))
    din("onescol", (128, 1))
    din("zeros", (128, 1056))
    din("dftc1", (TC1 * 128, LF1))
    din("dfts1", (TC1 * 128, LF1))
    din("idft1", (2 * NFC1 * 128, L1))
    din("dftc2", (TC2 * 128, LF2))
    din("dfts2", (TC2 * 128, LF2))
    din("idft2", (2 * NFC2 * 128, L2))

    out_d = nc.dram_tensor("out", [BC, CIN, PRED], F32, kind="ExternalOutput")

    # DRAM scratch
    enc_dram = nc.dram_tensor("enc_scratch", [BC, DC, 128, L1], MMDT)
    dec_dram = nc.dram_tensor("dec_scratch", [BC, DC, 128, L2], MMDT)
    tsum_dram = nc.dram_tensor("tsum_scratch", [BC, DC, 128, L2], F32)
    seas_dram = nc.dram_tensor("seas_scratch", [BC, CIN, LAB], MMDT)
    trendi_dram = nc.dram_tensor("trendi_scratch", [BC, CIN, L2], F32)

    with TileContext(nc) as tc, ExitStack() as top:
        cpool = top.enter_context(tc.tile_pool(name="consts", bufs=1))
        ones_row = cpool.tile([1, 512], MMDT)
        nc.sync.dma_start(ones_row, dram["ones512"][:])
        ones_col = cpool.tile([128, 1], MMDT)
        nc.sync.dma_start(ones_col, dram["onescol"][:])
        zeros_t = cpool.tile([128, 1056], MMDT)
        nc.sync.dma_start(zeros_t, dram["zeros"][:])

        # ---------------------------------------------------- helper closures
        def linear_T(ps_pool, out, X, W, bias, L, tcn):
            """out[128, tcn, 512] (seq-part) = X.T @ W + bias ; X[128,DC,L]."""
            if L % 128:
                nc.vector.tensor_copy(out[:, tcn - 1, :], zeros_t[:, 0:512])
            for mt in range(tcn):
                m = min(128, L - mt * 128)
                ps = ps_pool.tile([128, 512], F32, tag="mm")
                for kc in range(DC):
                    nc.tensor.matmul(ps[0:m, :], X[:, kc, mt * 128:mt * 128 + m],
                                     W[:, kc, :], start=(kc == 0), stop=False)
                nc.tensor.matmul(ps[0:m, :], ones_row[0:1, 0:m], bias,
                                 start=False, stop=True)
                nc.scalar.copy(out[0:m, mt, :], ps[0:m, :])

        def dft_S(ps_pool, tmp_pool, Sstk, q, k, dftc, dfts, fts, nfc, tck, tcq):
            """Sstk[128, 2*nfc, 1] f32r: stacked sum_c Qf*conj(Kf)."""
            nc.vector.tensor_copy(Sstk[:, :, 0], zeros_t[:, 0:Sstk.shape[1]])
            for ft, (f0, fm) in enumerate(fts):
                scr = {}
                for nm, mat, src, tcs in (("qr", dftc, q, tcq), ("qi", dfts, q, tcq),
                                          ("kr", dftc, k, tck), ("ki", dfts, k, tck)):
                    ps = ps_pool.tile([128, 512], F32, tag="pdft", bufs=2,
                                      name=f"pdft_{nm}")
                    for t in range(tcs):
                        nc.tensor.matmul(ps[0:fm, :], mat[:, t, f0:f0 + fm],
                                         src[:, t, :], start=(t == 0),
                                         stop=(t == tcs - 1))
                    sc = tmp_pool.tile([128, 512], F32, tag=f"s{nm}",
                                       name=f"s{nm}")
                    nc.scalar.copy(sc[0:fm, :], ps[0:fm, :])
                    scr[nm] = sc
                prod = tmp_pool.tile([128, 512], F32, tag="prod", bufs=2)
                cols = tmp_pool.tile([128, 4], F32, tag="cols", bufs=2)
                for ci, (xa, xb) in enumerate((("qr", "kr"), ("qi", "ki"),
                                               ("qi", "kr"), ("qr", "ki"))):
                    nc.vector.scalar_tensor_tensor(
                        prod[0:fm, :], scr[xa][0:fm, :], 1.0, scr[xb][0:fm, :],
                        op0=AL.bypass, op1=AL.mult,
                        accum_out=cols[0:fm, ci:ci + 1])
                nc.vector.tensor_tensor(Sstk[0:fm, ft, 0:1], cols[0:fm, 0:1],
                                        cols[0:fm, 1:2], AL.add)
                nc.vector.tensor_tensor(Sstk[0:fm, nfc + ft, 0:1], cols[0:fm, 2:3],
                                        cols[0:fm, 3:4], AL.subtract)

        def topk_tw(ps_pool, tmp_pool, Sstk, idft, nfc, L, nts):
            """corr -> (twb[128,8] f32, i8[1,8] u32)."""
            corr = tmp_pool.tile([1, L], F32, tag="corr")
            for nt, (n0, n) in enumerate(nts):
                psc = ps_pool.tile([1, 512], F32, tag="corrps")
                for j in range(2 * nfc):
                    nc.tensor.matmul(psc[:, 0:n], Sstk[:, j, 0:1],
                                     idft[:, j, n0:n0 + n],
                                     start=(j == 0), stop=(j == 2 * nfc - 1))
                nc.scalar.copy(corr[:, n0:n0 + n], psc[:, 0:n])
            w8 = tmp_pool.tile([1, 8], F32, tag="w8")
            i8 = tmp_pool.tile([1, 8], U32, tag="i8")
            nc.vector.max_with_indices(w8, i8, corr)
            e6 = tmp_pool.tile([1, 8], F32, tag="e6")
            nc.vector.memset(e6[:, TOPK:8], 0.0)
            nc.vector.tensor_scalar_sub(e6[:, 0:TOPK], w8[:, 0:TOPK], w8[:, 0:1])
            nc.scalar.activation(e6[:, 0:TOPK], e6[:, 0:TOPK], AF.Exp)
            ssum = tmp_pool.tile([1, 1], F32, tag="ssum")
            nc.vector.reduce_sum(ssum, e6[:, 0:TOPK], axis=AX.X)
            nc.vector.reciprocal(ssum, ssum)
            nc.vector.tensor_scalar_mul(e6[:, 0:TOPK], e6[:, 0:TOPK], ssum)
            twb = tmp_pool.tile([128, 8], F32, tag="twb")
            nc.gpsimd.partition_broadcast(twb, e6[0:1, :])
            return twb, i8

        def agg_delays(agg, vv, twb, i8, L):
            for kk in range(TOPK):
                dly = nc.values_load(i8[0:1, kk:kk + 1], min_val=0, max_val=L - 1,
                                     engines=[DVE], skip_runtime_bounds_check=True)
                for mc in range(DC):
                    src = vv[:, mc, bass.ds(dly, L)]
                    if kk == 0:
                        nc.vector.scalar_tensor_tensor(
                            agg[:, mc, :], src, twb[:, 0:1], src,
                            op0=AL.mult, op1=AL.bypass)
                    else:
                        nc.vector.scalar_tensor_tensor(
                            agg[:, mc, :], src, twb[:, kk:kk + 1], agg[:, mc, :],
                            op0=AL.mult, op1=AL.add)

        def out_proj_residual(ps_pool, X, agg, W, boC, nts):
            """X += agg.T@W + bo  (F-layout, in place)."""
            for mc in range(DC):
                for (n0, n) in nts:
                    ps = ps_pool.tile([128, 512], F32, tag="mm")
                    for kc in range(DC):
                        nc.tensor.matmul(ps[:, 0:n], W[:, kc, mc * 128:(mc + 1) * 128],
                                         agg[:, kc, n0:n0 + n],
                                         start=(kc == 0), stop=(kc == DC - 1))
                    nc.vector.scalar_tensor_tensor(
                        X[:, mc, n0:n0 + n], ps[:, 0:n], boC[:, mc:mc + 1],
                        X[:, mc, n0:n0 + n], op0=AL.add, op1=AL.add)

        def decomp(tmp_pool, X, L, chunks=DC, trend_to=None, trend_accum=None,
                   trend_dram_b=None, db=1):
            """X <- X - mavg(X) in place; optionally emit trend (mavg)."""
            for mc in range(chunks):
                xp = tmp_pool.tile([128, L + 2 * PAD], F32, tag="xp", bufs=db)
                nc.scalar.copy(xp[:, PAD:PAD + L], X[:, mc, :])
                nc.vector.tensor_copy(xp[:, 0:PAD],
                                      X[:, mc, 0:1].to_broadcast([128, PAD]))
                nc.vector.tensor_copy(xp[:, PAD + L:],
                                      X[:, mc, L - 1:L].to_broadcast([128, PAD]))
                cs = tmp_pool.tile([128, L + 2 * PAD + 1], F32, tag="cs", bufs=db)
                nc.vector.memset(cs[:, 0:1], 0.0)
                nc.vector.tensor_tensor_scan(cs[:, 1:], xp, xp, 0.0, AL.add, AL.bypass)
                dt = tmp_pool.tile([128, L], F32, tag="dt", bufs=2)
                nc.vector.tensor_tensor(dt, cs[:, MA:MA + L], cs[:, 0:L], AL.subtract)
                if trend_to is not None:
                    nc.vector.tensor_scalar_mul(trend_to[:, mc, :], dt, 1.0 / MA)
                if trend_accum is not None:
                    tt = tmp_pool.tile([128, L], F32, tag="taccum", bufs=2)
                    nc.vector.tensor_scalar_mul(tt, dt, 1.0 / MA)
                    nc.gpsimd.dma_start(tsum_dram[trend_dram_b, mc], tt,
                                        accum_op=(AL.add if trend_accum == "add"
                                                  else AL.bypass))
                nc.vector.scalar_tensor_tensor(X[:, mc, :], dt, -1.0 / MA,
                                               X[:, mc, :], op0=AL.mult, op1=AL.add)

        def ffn(ps_pool, tmp_pool, X, W1, b1C, W2, b2C, L, nts):
            """X += gelu(X@W1+b1)@W2+b2 in place (F-layout)."""
            h = tmp_pool.tile([128, DFC, L], MMDT, tag="h")
            for mh in range(DFC):
                for (n0, n) in nts:
                    ps = ps_pool.tile([128, 512], F32, tag="mm")
                    for kc in range(DC):
                        nc.tensor.matmul(ps[:, 0:n], W1[:, kc, mh * 128:(mh + 1) * 128],
                                         X[:, kc, n0:n0 + n],
                                         start=(kc == 0), stop=(kc == DC - 1))
                    nc.scalar.activation(h[:, mh, n0:n0 + n], ps[:, 0:n], GELU,
                                         bias=b1C[:, mh:mh + 1])
            for mc in range(DC):
                for (n0, n) in nts:
                    ps = ps_pool.tile([128, 512], F32, tag="mm")
                    for kh in range(DFC):
                        nc.tensor.matmul(ps[:, 0:n], W2[:, kh, mc * 128:(mc + 1) * 128],
                                         h[:, kh, n0:n0 + n],
                                         start=(kh == 0), stop=(kh == DFC - 1))
                    nc.vector.scalar_tensor_tensor(
                        X[:, mc, n0:n0 + n], ps[:, 0:n], b2C[:, mc:mc + 1],
                        X[:, mc, n0:n0 + n], op0=AL.add, op1=AL.add)

        def layernorm(ps_pool, tmp_pool, X, Xln, L, nts, wD, bD):
            """Xln = LN(X) over feature dim (partition dim, DC chunks)."""
            xsq = tmp_pool.tile([128, DC, L], MMDT, tag="xsq")
            for mc in range(DC):
                nc.scalar.activation(xsq[:, mc, :], X[:, mc, :], AF.Square)
            pmu = ps_pool.tile([1, len(nts), 512], F32, tag="pmu")
            psq = ps_pool.tile([1, len(nts), 512], F32, tag="psq")
            for nt, (n0, n) in enumerate(nts):
                for kc in range(DC):
                    st, sp = (kc == 0), (kc == DC - 1)
                    nc.tensor.matmul(pmu[:, nt, 0:n], ones_col, X[:, kc, n0:n0 + n],
                                     start=st, stop=sp)
                    nc.tensor.matmul(psq[:, nt, 0:n], ones_col, xsq[:, kc, n0:n0 + n],
                                     start=st, stop=sp)
            stats = tmp_pool.tile([1, 2 * L], F32, tag="stats")
            mu, rstd = stats[:, 0:L], stats[:, L:2 * L]
            for nt, (n0, n) in enumerate(nts):
                nc.vector.tensor_scalar_mul(mu[:, n0:n0 + n], pmu[:, nt, 0:n], 1.0 / D)
                nc.vector.tensor_scalar_mul(rstd[:, n0:n0 + n], psq[:, nt, 0:n], 1.0 / D)
            musq = tmp_pool.tile([1, L], F32, tag="musq")
            nc.vector.tensor_tensor(musq, mu, mu, AL.mult)
            nc.vector.tensor_tensor(rstd, rstd, musq, AL.subtract)
            nc.vector.tensor_scalar_add(rstd, rstd, 1e-5)
            nc.scalar.activation(rstd, rstd, AF.Sqrt)
            nc.vector.reciprocal(rstd, rstd)
            stb = tmp_pool.tile([128, 2 * L], F32, tag="stb")
            nc.gpsimd.partition_broadcast(stb, stats[0:1, :])
            t = tmp_pool.tile([128, L], F32, tag="lnt")
            for mc in range(DC):
                nc.vector.tensor_tensor(t, X[:, mc, :], stb[:, 0:L], AL.subtract)
                nc.vector.tensor_tensor(t, t, stb[:, L:2 * L], AL.mult)
                nc.vector.tensor_scalar_mul(t, t, wD[:, mc:mc + 1])
                nc.vector.tensor_scalar_add(Xln[:, mc, :], t, bD[:, mc:mc + 1])

        # ======================================================== ENCODER
        with tc.tile_pool(name="acts", bufs=1) as apool:
            enc_acts = [apool.tile([128, DC, L1], MMDT, tag=f"enc{b}",
                                   name=f"enc_acts{b}")
                        for b in range(BC)]

            # ---- P0: embedding + init decomposition
            with nc.named_scope("P0_embed"), \
                 tc.tile_pool(name="p0t", bufs=2) as tp, \
                 tc.tile_pool(name="p0w", bufs=1) as wp, \
                 tc.tile_pool(name="p0ps", bufs=4, space="PSUM") as psp:
                embW = wp.tile([67, D], MMDT)
                nc.sync.dma_start(embW, dram["embW_e"][:])
                for b in range(BC):
                    win = tp.tile([67, L1], MMDT, tag="win")
                    nc.sync.dma_start(win[0:CIN, 1:L1], dram["xT"][b, :, 0:L1 - 1])
                    nc.sync.dma_start(win[0:CIN, 0:1], dram["xT"][b, :, L1 - 1:L1])
                    nc.sync.dma_start(win[CIN:2 * CIN, :], dram["xT"][b])
                    nc.sync.dma_start(win[2 * CIN:3 * CIN, 0:L1 - 1],
                                      dram["xT"][b, :, 1:L1])
                    nc.sync.dma_start(win[2 * CIN:3 * CIN, L1 - 1:L1],
                                      dram["xT"][b, :, 0:1])
                    nc.sync.dma_start(win[63:67, :], dram["mkeT"][b])
                    for mc in range(DC):
                        for (n0, n) in NT1:
                            ps = psp.tile([128, 512], F32, tag="mm")
                            nc.tensor.matmul(ps[:, 0:n],
                                             embW[:, mc * 128:(mc + 1) * 128],
                                             win[:, n0:n0 + n], start=True, stop=True)
                            nc.scalar.copy(enc_acts[b][:, mc, n0:n0 + n], ps[:, 0:n])

                # init series_decomp of x_enc (packed [84, .])
                xe = tp.tile([84, L1], MMDT, tag="xe")
                for b in range(BC):
                    nc.sync.dma_start(xe[b * CIN:(b + 1) * CIN, :], dram["xT"][b])
                xp = tp.tile([84, L1 + 2 * PAD], F32, tag="ixp")
                nc.scalar.copy(xp[:, PAD:PAD + L1], xe)
                nc.vector.tensor_copy(xp[:, 0:PAD], xe[:, 0:1].to_broadcast([84, PAD]))
                nc.vector.tensor_copy(xp[:, PAD + L1:],
                                      xe[:, L1 - 1:L1].to_broadcast([84, PAD]))
                cs = tp.tile([84, L1 + 2 * PAD + 1], F32, tag="ics")
                nc.vector.memset(cs[:, 0:1], 0.0)
                nc.vector.tensor_tensor_scan(cs[:, 1:], xp, xp, 0.0, AL.add, AL.bypass)
                dt = tp.tile([84, L1], F32, tag="idt")
                nc.vector.tensor_tensor(dt, cs[:, MA:MA + L1], cs[:, 0:L1], AL.subtract)
                seas = tp.tile([84, L1], MMDT, tag="iseas")
                nc.vector.scalar_tensor_tensor(seas, dt, -1.0 / MA, xe,
                                               op0=AL.mult, op1=AL.add)
                trend = tp.tile([84, L1], F32, tag="itrend")
                nc.vector.tensor_scalar_mul(trend, dt, 1.0 / MA)
                mean = tp.tile([84, 1], F32, tag="imean")
                nc.vector.reduce_sum(mean, xe, axis=AX.X)
                nc.vector.tensor_scalar_mul(mean, mean, 1.0 / L1)
                meanb = tp.tile([84, PRED], F32, tag="imeanb")
                nc.vector.tensor_copy(meanb, mean.to_broadcast([84, PRED]))
                for b in range(BC):
                    sl = slice(b * CIN, (b + 1) * CIN)
                    nc.sync.dma_start(seas_dram[b], seas[sl, L1 - LAB:L1])
                    nc.sync.dma_start(trendi_dram[b, :, 0:LAB], trend[sl, L1 - LAB:L1])
                    nc.sync.dma_start(trendi_dram[b, :, LAB:L2], meanb[sl, :])

            # ---- P1/P2: encoder layers
            if True:
                for l in range(EL):
                    with nc.named_scope(f"enc{l}_att"), \
                         tc.tile_pool(name="dft1", bufs=1) as dft1p, \
                         tc.tile_pool(name="eatw", bufs=1) as wp, \
                         tc.tile_pool(name="eatt", bufs=1) as tp, \
                         tc.tile_pool(name="eatps", bufs=1, space="PSUM") as psp, \
                         tc.tile_pool(name="eatps2", bufs=4, space="PSUM") as psp2:
                        dftc1 = dft1p.tile([128, TC1, LF1], MMDT, tag="dftc1")
                        dfts1 = dft1p.tile([128, TC1, LF1], MMDT, tag="dfts1")
                        idft1 = dft1p.tile([128, 2 * NFC1, L1], MMDT, tag="idft1")
                        nc.sync.dma_start(dftc1, dram["dftc1"][:].rearrange(
                            "(c p) f -> p c f", p=128))
                        nc.sync.dma_start(dfts1, dram["dfts1"][:].rearrange(
                            "(c p) f -> p c f", p=128))
                        nc.sync.dma_start(idft1, dram["idft1"][:].rearrange(
                            "(c p) f -> p c f", p=128))
                        Ws = {}
                        for nm in ("q", "k", "v", "o"):
                            Ws[nm] = wp.tile([128, DC, D], MMDT, tag=f"W{nm}",
                                             name=f"W{nm}")
                            nc.sync.dma_start(Ws[nm], dram[f"eW{nm}{l}"][:].rearrange(
                                "(c p) f -> p c f", p=128))
                        bq = wp.tile([1, D], MMDT, tag="bq")
                        bk = wp.tile([1, D], MMDT, tag="bk")
                        nc.sync.dma_start(bq, dram[f"ebq{l}"][:])
                        nc.sync.dma_start(bk, dram[f"ebk{l}"][:])
                        bvC = wp.tile([128, DC], F32, tag="bvC")
                        boC = wp.tile([128, DC], F32, tag="boC")
                        nc.sync.dma_start(bvC, dram[f"ebvC{l}"][:])
                        nc.sync.dma_start(boC, dram[f"eboC{l}"][:])
                        for b in range(BC):
                            X = enc_acts[b]
                            q = tp.tile([128, TC1, 512], MMDT, tag="q")
                            k = tp.tile([128, TC1, 512], MMDT, tag="k", bufs=2)
                            linear_T(psp2, q, X, Ws["q"], bq, L1, TC1)
                            linear_T(psp2, k, X, Ws["k"], bk, L1, TC1)
                            Sstk = tp.tile([128, 2 * NFC1, 1], MMDT, tag="Sstk")
                            dft_S(psp, tp, Sstk, q, k, dftc1, dfts1, FT1, NFC1,
                                  TC1, TC1)
                            twb, i8 = topk_tw(psp, tp, Sstk, idft1, NFC1, L1, NT1)
                            vv = tp.tile([128, DC, 2 * L1], F32, tag="q")
                            for mc in range(DC):
                                for (n0, n) in NT1:
                                    ps = psp2.tile([128, 512], F32, tag="mm")
                                    for kc in range(DC):
                                        nc.tensor.matmul(
                                            ps[:, 0:n],
                                            Ws["v"][:, kc, mc * 128:(mc + 1) * 128],
                                            X[:, kc, n0:n0 + n],
                                            start=(kc == 0), stop=(kc == DC - 1))
                                    nc.scalar.activation(vv[:, mc, n0:n0 + n],
                                                         ps[:, 0:n], AF.Identity,
                                                         bias=bvC[:, mc:mc + 1])
                                    nc.scalar.activation(vv[:, mc, L1 + n0:L1 + n0 + n],
                                                         ps[:, 0:n], AF.Identity,
                                                         bias=bvC[:, mc:mc + 1])
                            agg = tp.tile([128, DC, L1], MMDT, tag="k", bufs=2)
                            agg_delays(agg, vv, twb, i8, L1)
                            out_proj_residual(psp2, X, agg, Ws["o"], boC, NT1)
                            decomp(tp, X, L1)

                    with nc.named_scope(f"enc{l}_ffn"), \
                         tc.tile_pool(name="effw", bufs=1) as wp, \
                         tc.tile_pool(name="efft", bufs=1) as tp, \
                         tc.tile_pool(name="effps", bufs=6, space="PSUM") as psp:
                        W1 = wp.tile([128, DC, DFF], MMDT, tag="W1")
                        W2 = wp.tile([128, DFC, D], MMDT, tag="W2")
                        nc.sync.dma_start(W1, dram[f"eW1{l}"][:].rearrange(
                            "(c p) f -> p c f", p=128))
                        nc.sync.dma_start(W2, dram[f"eW2{l}"][:].rearrange(
                            "(c p) f -> p c f", p=128))
                        b1C = wp.tile([128, DFC], F32, tag="b1C")
                        b2C = wp.tile([128, DC], F32, tag="b2C")
                        nc.sync.dma_start(b1C, dram[f"eb1{l}"][:])
                        nc.sync.dma_start(b2C, dram[f"eb2{l}"][:])
                        for b in range(BC):
                            ffn(psp, tp, enc_acts[b], W1, b1C, W2, b2C, L1, NT1)
                            decomp(tp, enc_acts[b], L1, db=2)

            # ---- P3: final encoder LN -> enc_dram
            with nc.named_scope("enc_ln"), \
                 tc.tile_pool(name="lnt", bufs=1) as tp, \
                 tc.tile_pool(name="lnw", bufs=1) as wp, \
                 tc.tile_pool(name="lnps", bufs=1, space="PSUM") as psp:
                enW = wp.tile([128, DC], F32, tag="enW")
                enB = wp.tile([128, DC], F32, tag="enB")
                nc.sync.dma_start(enW, dram["enW"][:])
                nc.sync.dma_start(enB, dram["enB"][:])
                for b in range(BC):
                    xln = tp.tile([128, DC, L1], MMDT, tag="xln")
                    layernorm(psp, tp, enc_acts[b], xln, L1, NT1, enW, enB)
                    for mc in range(DC):
                        nc.sync.dma_start(enc_dram[b, mc], xln[:, mc, :])

        # ======================================================== DECODER
        # ---- P4: decoder embedding -> dec_dram
        with nc.named_scope("dec_embed"), \
             tc.tile_pool(name="p4t", bufs=2) as tp, \
             tc.tile_pool(name="p4w", bufs=1) as wp, \
             tc.tile_pool(name="p4ps", bufs=4, space="PSUM") as psp:
            embW = wp.tile([67, D], MMDT)
            nc.sync.dma_start(embW, dram["embW_d"][:])
            for b in range(BC):
                win = tp.tile([67, L2], MMDT, tag="win2")
                nc.vector.tensor_copy(win[0:63, :], zeros_t[0:63, 0:L2])
                nc.sync.dma_start(win[0:CIN, 1:LAB + 1], seas_dram[b])
                nc.sync.dma_start(win[CIN:2 * CIN, 0:LAB], seas_dram[b])
                nc.sync.dma_start(win[2 * CIN:3 * CIN, 0:LAB - 1],
                                  seas_dram[b, :, 1:LAB])
                nc.sync.dma_start(win[2 * CIN:3 * CIN, L2 - 1:L2],
                                  seas_dram[b, :, 0:1])
                nc.sync.dma_start(win[63:67, :], dram["mkdT"][b])
                for mc in range(DC):
                    for (n0, n) in NT2:
                        ps = psp.tile([128, 512], F32, tag="mm")
                        nc.tensor.matmul(ps[:, 0:n], embW[:, mc * 128:(mc + 1) * 128],
                                         win[:, n0:n0 + n], start=True, stop=True)
                        xpart = tp.tile([128, 512], MMDT, tag="xpart")
                        nc.scalar.copy(xpart[:, 0:n], ps[:, 0:n])
                        nc.sync.dma_start(dec_dram[b, mc, :, n0:n0 + n], xpart[:, 0:n])

        # ---- P5/P6: decoder attentions
        for phase, pre in (("self", "ds"), ("cross", "dc")):
            with nc.named_scope(f"dec_{phase}"), \
                 tc.tile_pool(name="datw", bufs=1) as wp, \
                 tc.tile_pool(name="datt", bufs=1) as tp, \
                 tc.tile_pool(name="dft2", bufs=1) as dp, \
                 tc.tile_pool(name="datps", bufs=1, space="PSUM") as psp, \
                 tc.tile_pool(name="datps2", bufs=4, space="PSUM") as psp2:
                Ws = {}
                for nm in ("q", "k", "v", "o"):
                    Ws[nm] = wp.tile([128, DC, D], MMDT, tag=f"W{nm}",
                                     name=f"W{nm}")
                    nc.sync.dma_start(Ws[nm], dram[f"{pre}W{nm}"][:].rearrange(
                        "(c p) f -> p c f", p=128))
                bq = wp.tile([1, D], MMDT, tag="bq")
                bk = wp.tile([1, D], MMDT, tag="bk")
                nc.sync.dma_start(bq, dram[f"{pre}bq"][:])
                nc.sync.dma_start(bk, dram[f"{pre}bk"][:])
                bvC = wp.tile([128, DC], F32, tag="bvC")
                boC = wp.tile([128, DC], F32, tag="boC")
                nc.sync.dma_start(bvC, dram[f"{pre}bvC"][:])
                nc.sync.dma_start(boC, dram[f"{pre}boC"][:])
                kvsrc_chunks = TC2 if phase == "self" else TC1
                for b in range(BC):
                    X = tp.tile([128, DC, L2], MMDT, tag="Xd")
                    for mc in range(DC):
                        nc.sync.dma_start(X[:, mc, :], dec_dram[b, mc])
                    if phase == "self":
                        KV = X
                    else:
                        KV = tp.tile([128, DC, L1], MMDT, tag="KV")
                        for mc in range(DC):
                            nc.sync.dma_start(KV[:, mc, :], enc_dram[b, mc])
                    q = tp.tile([128, TC2, 512], MMDT, tag="q2")
                    k = tp.tile([128, TC2, 512], MMDT, tag="k2")
                    linear_T(psp2, q, X, Ws["q"], bq, L2, TC2)
                    linear_T(psp2, k, KV, Ws["k"], bk,
                             L2 if phase == "self" else L1, kvsrc_chunks)
                    dftcs = dp.tile([128, TC2, 2 * LF2], MMDT, tag="dftbig")
                    nc.sync.dma_start(dftcs[:, :, 0:LF2], dram["dftc2"][:].rearrange(
                        "(c p) f -> p c f", p=128))
                    nc.sync.dma_start(dftcs[:, :, LF2:], dram["dfts2"][:].rearrange(
                        "(c p) f -> p c f", p=128))
                    Sstk = tp.tile([128, 2 * NFC2, 1], MMDT, tag="Sstk2")
                    dft_S(psp, tp, Sstk, q, k, dftcs[:, :, 0:LF2],
                          dftcs[:, :, LF2:2 * LF2], FT2, NFC2, kvsrc_chunks, TC2)
                    idft2 = dp.tile([128, 2 * NFC2, L2], MMDT, tag="dftbig")
                    nc.sync.dma_start(idft2, dram["idft2"][:].rearrange(
                        "(c p) f -> p c f", p=128))
                    twb, i8 = topk_tw(psp, tp, Sstk, idft2, NFC2, L2, NT2)
                    VL = 2 * L2
                    vv = tp.tile([128, DC, VL], F32, tag="q2")
                    if phase == "cross":
                        nc.vector.memset(vv, 0.0)
                    kvL = L2 if phase == "self" else L1
                    kvNT = NT2 if phase == "self" else NT1
                    for mc in range(DC):
                        for (n0, n) in kvNT:
                            ps = psp2.tile([128, 512], F32, tag="mm")
                            for kc in range(DC):
                                nc.tensor.matmul(
                                    ps[:, 0:n],
                                    Ws["v"][:, kc, mc * 128:(mc + 1) * 128],
                                    KV[:, kc, n0:n0 + n],
                                    start=(kc == 0), stop=(kc == DC - 1))
                            nc.scalar.activation(vv[:, mc, n0:n0 + n], ps[:, 0:n],
                                                 AF.Identity, bias=bvC[:, mc:mc + 1])
                            nc.scalar.activation(vv[:, mc, L2 + n0:L2 + n0 + n],
                                                 ps[:, 0:n], AF.Identity,
                                                 bias=bvC[:, mc:mc + 1])
                    agg = tp.tile([128, DC, L2], MMDT, tag="k2")
                    agg_delays(agg, vv, twb, i8, L2)
                    out_proj_residual(psp2, X, agg, Ws["o"], boC, NT2)
                    decomp(tp, X, L2, trend_accum=("bypass" if phase == "self"
                                                   else "add"), trend_dram_b=b)
                    for mc in range(DC):
                        nc.sync.dma_start(dec_dram[b, mc], X[:, mc, :])

        # ---- P7: decoder FFN
        with nc.named_scope("dec_ffn"), \
             tc.tile_pool(name="dffw", bufs=1) as wp, \
             tc.tile_pool(name="dfft", bufs=1) as tp, \
             tc.tile_pool(name="dffps", bufs=6, space="PSUM") as psp:
            W1 = wp.tile([128, DC, DFF], MMDT, tag="W1")
            W2 = wp.tile([128, DFC, D], MMDT, tag="W2")
            nc.sync.dma_start(W1, dram["dW1"][:].rearrange("(c p) f -> p c f", p=128))
            nc.sync.dma_start(W2, dram["dW2"][:].rearrange("(c p) f -> p c f", p=128))
            b1C = wp.tile([128, DFC], F32, tag="b1C")
            b2C = wp.tile([128, DC], F32, tag="b2C")
            nc.sync.dma_start(b1C, dram["db1"][:])
            nc.sync.dma_start(b2C, dram["db2"][:])
            for b in range(BC):
                X = tp.tile([128, DC, L2], MMDT, tag="Xd")
                for mc in range(DC):
                    nc.sync.dma_start(X[:, mc, :], dec_dram[b, mc])
                ffn(psp, tp, X, W1, b1C, W2, b2C, L2, NT2)
                decomp(tp, X, L2, trend_accum="add", trend_dram_b=b, db=2)
                for mc in range(DC):
                    nc.sync.dma_start(dec_dram[b, mc], X[:, mc, :])

        # ---- P8: final (LN + proj, trend conv, combine)
        with nc.named_scope("final"), \
             tc.tile_pool(name="fint", bufs=1) as tp, \
             tc.tile_pool(name="finw", bufs=1) as wp, \
             tc.tile_pool(name="finps", bufs=1, space="PSUM") as psp, \
             tc.tile_pool(name="finps2", bufs=2, space="PSUM") as psp2:
            trendW = wp.tile([128, 3 * DC, CIN], MMDT, tag="trendW")
            nc.sync.dma_start(trendW, dram["trendW"][:].rearrange(
                "(c p) f -> p c f", p=128))
            projW = wp.tile([128, DC, CIN], MMDT, tag="projW")
            nc.sync.dma_start(projW, dram["projW"][:].rearrange(
                "(c p) f -> p c f", p=128))
            projB = wp.tile([CIN, 1], F32, tag="projB")
            nc.sync.dma_start(projB, dram["projB"][:])
            dnW = wp.tile([128, DC], F32, tag="dnW")
            dnB = wp.tile([128, DC], F32, tag="dnB")
            nc.sync.dma_start(dnW, dram["dnW"][:])
            nc.sync.dma_start(dnB, dram["dnB"][:])
            for b in range(BC):
                X = tp.tile([128, DC, L2], MMDT, tag="Xd")
                for mc in range(DC):
                    nc.sync.dma_start(X[:, mc, :], dec_dram[b, mc])
                xln = tp.tile([128, DC, L2], MMDT, tag="xln2")
                layernorm(psp, tp, X, xln, L2, NT2, dnW, dnB)
                seasonal = tp.tile([CIN, L2], F32, tag="seasonal")
                for (n0, n) in NT2:
                    ps = psp2.tile([CIN, 512], F32, tag="sm")
                    for kc in range(DC):
                        nc.tensor.matmul(ps[:, 0:n], projW[:, kc, :],
                                         xln[:, kc, n0:n0 + n],
                                         start=(kc == 0), stop=(kc == DC - 1))
                    nc.scalar.activation(seasonal[:, n0:n0 + n], ps[:, 0:n],
                                         AF.Identity, bias=projB)
                # trend conv windows [12 chunks, L2] circular
                tsum = tp.tile([128, DC, L2], F32, tag="tsum")
                for mc in range(DC):
                    nc.sync.dma_start(tsum[:, mc, :], tsum_dram[b, mc])
                winT = tp.tile([128, 3 * DC, L2], MMDT, tag="winT")
                for mc in range(DC):
                    # j=0: shift -1 ; j=1: center ; j=2: shift +1 (circular)
                    nc.scalar.copy(winT[:, mc, 1:L2], tsum[:, mc, 0:L2 - 1])
                    nc.scalar.copy(winT[:, mc, 0:1], tsum[:, mc, L2 - 1:L2])
                    nc.scalar.copy(winT[:, DC + mc, :], tsum[:, mc, :])
                    nc.scalar.copy(winT[:, 2 * DC + mc, 0:L2 - 1], tsum[:, mc, 1:L2])
                    nc.scalar.copy(winT[:, 2 * DC + mc, L2 - 1:L2], tsum[:, mc, 0:1])
                trendi = tp.tile([CIN, L2], F32, tag="trendi")
                nc.sync.dma_start(trendi, trendi_dram[b])
                outt = tp.tile([CIN, PRED], F32, tag="outt")
                for nt, (n0, n) in enumerate(NT2):
                    ps = psp2.tile([CIN, 512], F32, tag="sm")
                    for j in range(3 * DC):
                        nc.tensor.matmul(ps[:, 0:n], trendW[:, j, :],
                                         winT[:, j, n0:n0 + n],
                                         start=(j == 0), stop=(j == 3 * DC - 1))
                    trend_sl = tp.tile([CIN, 512], F32, tag="trend_sl")
                    nc.vector.tensor_tensor(trend_sl[:, 0:n], ps[:, 0:n],
                                            trendi[:, n0:n0 + n], AL.add)
                    # add seasonal, write PRED slice (cols >= LAB)
                    lo = max(n0, LAB)
                    hi = n0 + n
                    if hi > lo:
                        nc.vector.tensor_tensor(
                            outt[:, lo - LAB:hi - LAB], trend_sl[:, lo - n0:hi - n0],
                            seasonal[:, lo:hi], AL.add)
                nc.sync.dma_start(out_d[b], outt)

    nc.compile()
    return nc


# ---------------------------------------------------------------- entry point
_CACHE = {}
LAST_EXEC_NS = [None]
SHARDED_INPUTS = ("xT", "mkeT", "mkdT")


def _get_executor(use_f32r=True):
    if "run" in _CACHE:
        return _CACHE["run"]
    import jax
    from jax.sharding import Mesh, PartitionSpec
    try:
        from jax.experimental.shard_map import shard_map
    except Exception:
        from jax.shard_map import shard_map
    from concourse import bass2jax

    bass2jax.install_neuronx_cc_hook()
    nc = build_nc(use_f32r=use_f32r)
    _CACHE["nc"] = nc

    part_name = (nc.partition_id_tensor.name
                 if nc.partition_id_tensor else None)
    in_names, out_names, out_avals = [], [], []
    for alloc in nc.m.functions[0].allocations:
        if not isinstance(alloc, mybir.MemoryLocationSet):
            continue
        name = alloc.memorylocations[0].name
        if alloc.kind == "ExternalInput":
            if name != part_name:
                in_names.append(name)
        elif alloc.kind == "ExternalOutput":
            out_names.append(name)
            out_avals.append(jax.core.ShapedArray(
                tuple(alloc.tensor_shape), mybir.dt.np(alloc.dtype)))
    n_params = len(in_names)
    n_outs = len(out_names)
    all_names = list(in_names) + list(out_names)
    if part_name is not None:
        all_names.append(part_name)

    donate = tuple(range(n_params, n_params + n_outs))

    def _body(*args):
        operands = list(args)
        if part_name is not None:
            operands.append(bass2jax.partition_id_tensor())
        outs = bass2jax._bass_exec_p.bind(
            *operands,
            out_avals=tuple(out_avals),
            in_names=tuple(all_names),
            out_names=tuple(out_names),
            lowering_input_output_aliases=(),
            sim_require_finite=True,
            sim_require_nnan=True,
            nc=nc,
        )
        return tuple(outs)

    devices = [d for d in jax.devices() if d.platform != "cpu"][:NCORES]
    if len(devices) < NCORES:
        devices = jax.devices()[:NCORES]
    assert len(devices) == NCORES, f"need {NCORES} neuron cores"
    mesh = Mesh(np.asarray(devices), ("core",))

    def spec(name):
        return (PartitionSpec("core") if name in SHARDED_INPUTS
                else PartitionSpec())

    in_specs = tuple(spec(n) for n in in_names) + \
        (PartitionSpec("core"),) * n_outs
    out_specs = (PartitionSpec("core"),) * n_outs
    sharded = jax.jit(
        shard_map(_body, mesh=mesh, in_specs=in_specs, out_specs=out_specs,
                  check_rep=False),
        donate_argnums=donate, keep_unused=True)

    def run(per_core_maps):
        args = []
        for name in in_names:
            if name in SHARDED_INPUTS:
                args.append(np.concatenate(
                    [m[name] for m in per_core_maps], 0))
            else:
                args.append(np.asarray(per_core_maps[0][name]))
        zeros = [np.zeros((NCORES * a.shape[0], *a.shape[1:]), a.dtype)
                 for a in (np.zeros(s.shape, s.dtype) for s in out_avals)]
        outs = sharded(*args, *zeros)
        res = []
        for c in range(NCORES):
            res.append({name: np.asarray(outs[i]).reshape(
                NCORES, *out_avals[i].shape)[c]
                for i, name in enumerate(out_names)})
        return res

    _CACHE["run"] = run
    _CACHE["sharded"] = sharded
    _CACHE["in_names"] = in_names
    _CACHE["out_avals"] = out_avals
    return run


def kernel(**inputs):
    use_f32r = os.environ.get("AK_F32", "") != "1"
    per_core = host_prep(inputs)
    run = _get_executor(use_f32r=use_f32r)
    results = run(per_core)
    outs = [np.transpose(r["out"], (0, 2, 1)) for r in results]
    return np.concatenate(outs, 0).astype(np.float32)


if __name__ == "__main__":
    pass
